# revision 2
# baseline (speedup 1.0000x reference)
"""Trainium2 Bass kernel for a OneBlob-encoded 3-layer MLP (ConditioningNetwork).

Math:  x = clip(concat(pos01, wi01, rough01), 0, 1)          [N, 7]
       enc[n, d*32+j] = exp(-0.5 ((x[n,d]-c[j]) / sigma)^2)  [N, 224], sigma = 1/32
       y = relu(relu(enc@W1+b1)@W2+b2)@W3+b3                 [N, 64]

Strategy (pure data parallel over 8 cores, weights replicated):
  - The Gaussian exponent z = -(x-c)^2/(2 sigma^2) is affine in (x, x^2), so it is
    computed on the PE as one small matmul ("expand"):  z = L^T @ [x; x^2; 1].
    The quadratic has catastrophic cancellation near x == c, so x and x^2 are fed
    as exact fp16 hi+lo pairs and L carries hi/lo weight rows; all products are
    exact in fp16 (accumulated fp32 in PSUM) giving |dz| ~ 1e-3.
  - enc = Exp(z) on the scalar engine (ACT) straight out of PSUM, fp16 into SBUF.
  - Per 512-ray half, z is packed [128, 1024]: cols 0:512 = enc rows 0:128,
    cols 512:1024 = enc rows 128:224 (+ 32 benign pad rows so all 128 PSUM
    partitions are written). One ACT Exp op covers the whole half -> no idle
    ACT lanes and half-granular z double-buffering (2 tiles x 2 banks).
  - 3 MLP matmuls in fp16 (fp32 PSUM). A/B = consecutive 512-ray halves of a
    1024-ray supertile: L1 col-packed (psum partitions 0:64 / 64:128), L2/L3
    quadrant-packed (disjoint row+col groups -> concurrent on the PE).
    Bias+ReLU / bias+cast are single DVE tensor_scalar ops per [128, 512] tile.
  - The PE's HAM clock gate never warms on row/col-masked (tile_position)
    matmuls -- measured: they run at 1.2 GHz forever, but do run at 2.4 GHz
    once warm. So the kernel issues an unmasked full-array warm-up burst at
    start and one tiny unmasked dummy matmul per supertile (output is
    overwritten by L1's start accumulation) to keep the HAM at K=8/8.
  - Output is produced feature-major, packed [128, Nc/2] fp16 (per supertile:
    A-half rays on partitions 64:128, B-half on 0:64 -- L3 quadrant flip), and
    unpacked/transposed/cast on the host.

Input row packing (fp16, 37 rows, shipped twice: rows 0:37 -> SBUF partitions
0:37 for the hi expand matmul, rows 37:74 -> partitions 64:101 for the lo one
so the two run concurrently on disjoint PE row groups):
  rows  0: 7  x_hi          (weights: w_hi,  w = c/sigma^2 per enc row)
  rows  7:14  x_hi (dup)    (weights: w_lo = w - fp16(w))
  rows 14:21  x_lo          (weights: w_hi)
  rows 21:28  q_hi, q = x^2 (weights: -1/(2 sigma^2), exact in fp16)
  rows 28:35  q_lo          (weights: -1/(2 sigma^2))
  row  35     ones          (weights: u_hi, u = -c^2/(2 sigma^2))
  row  36     ones          (weights: u_lo)
"""

import sys

import numpy as np

if "/opt/trn_rl_repo" not in sys.path:
    sys.path.insert(0, "/opt/trn_rl_repo")

N_CORES = 8
N_TOTAL = 1048576
NC_RAYS = N_TOTAL // N_CORES  # 131072 rays per core
BINS = 32
HID = 64
OUT = 64
IN_DIMS = 7
ENC = IN_DIMS * BINS  # 224
SIGMA = 1.0 / BINS

KROWS = 37  # packed input rows (see module docstring)
B = 512  # rays per matmul (one fp32 PSUM bank)
SUPER = 2 * B  # rays per supertile (A/B halves)
G = 8  # supertiles per DMA group
GROUP_RAYS = SUPER * G  # 8192
N_GROUPS = NC_RAYS // GROUP_RAYS  # 16

# Set by the last kernel() call so a test harness can read profile/exec time.
LAST_RESULTS = None

_BUILD_CACHE = {}


def _build_bass(nc_rays, n_groups):
    import concourse.tile as tile
    from concourse import bacc, mybir

    dt = mybir.dt
    Act = mybir.ActivationFunctionType
    Alu = mybir.AluOpType

    nc = bacc.Bacc("TRN2", target_bir_lowering=False, debug=False)

    n_super = n_groups * G

    xp = nc.dram_tensor("xp", [2 * KROWS, nc_rays], dt.float16,
                        kind="ExternalInput")
    lw = nc.dram_tensor("lw", [KROWS, 256], dt.float16, kind="ExternalInput")
    w1a = nc.dram_tensor("w1a", [128, HID], dt.float16, kind="ExternalInput")
    w1b = nc.dram_tensor("w1b", [ENC - 128, HID], dt.float16, kind="ExternalInput")
    w2s = nc.dram_tensor("w2s", [128, HID], dt.float16, kind="ExternalInput")
    w3s = nc.dram_tensor("w3s", [128, OUT], dt.float16, kind="ExternalInput")
    b1s = nc.dram_tensor("b1s", [128, 1], dt.float32, kind="ExternalInput")
    b2s = nc.dram_tensor("b2s", [128, 1], dt.float32, kind="ExternalInput")
    b3s = nc.dram_tensor("b3s", [128, 1], dt.float32, kind="ExternalInput")
    # Output, packed fp16: per 512-col supertile block, rows 64:128 = A-half
    # rays, rows 0:64 = B-half rays (L3's flipped quadrants).
    yt = nc.dram_tensor("yt", [128, nc_rays // 2], dt.float16,
                        kind="ExternalOutput")

    with tile.TileContext(nc) as tc:
        with (
            tc.tile_pool(name="consts", bufs=1) as consts,
            tc.tile_pool(name="xpool", bufs=3) as xpool,
            tc.tile_pool(name="encp", bufs=4) as encp,
            tc.tile_pool(name="hp", bufs=3) as hp,
            tc.tile_pool(name="outp", bufs=2) as outp,
            tc.tile_pool(name="pz0", bufs=1, space="PSUM") as pz0,
            tc.tile_pool(name="pz1", bufs=1, space="PSUM") as pz1,
            tc.tile_pool(name="ph", bufs=4, space="PSUM") as ph,
        ):
            # lw replicated at partitions 0:37 (hi block) and 64:101 (lo block)
            lw2_t = consts.tile([101, 256], dt.float16, tag="lw2_t")
            nc.sync.dma_start(out=lw2_t[0:KROWS, :], in_=lw[:])
            nc.sync.dma_start(out=lw2_t[64 : 64 + KROWS, :], in_=lw[:])
            w1a_t = consts.tile([128, HID], dt.float16, tag="w1a_t")
            nc.sync.dma_start(out=w1a_t[:], in_=w1a[:])
            w1b_t = consts.tile([ENC - 128, HID], dt.float16, tag="w1b_t")
            nc.sync.dma_start(out=w1b_t[:], in_=w1b[:])
            w2s_t = consts.tile([128, HID], dt.float16, tag="w2s_t")
            nc.sync.dma_start(out=w2s_t[:], in_=w2s[:])
            w3s_t = consts.tile([128, OUT], dt.float16, tag="w3s_t")
            nc.sync.dma_start(out=w3s_t[:], in_=w3s[:])
            b1s_t = consts.tile([128, 1], dt.float32, tag="b1s_t")
            nc.sync.dma_start(out=b1s_t[:], in_=b1s[:])
            b2s_t = consts.tile([128, 1], dt.float32, tag="b2s_t")
            nc.sync.dma_start(out=b2s_t[:], in_=b2s[:])
            b3s_t = consts.tile([128, 1], dt.float32, tag="b3s_t")
            nc.sync.dma_start(out=b3s_t[:], in_=b3s[:])

            # Zero tiles driving the unmasked HAM warm-up matmuls.
            warm_w = consts.tile([128, 128], dt.float16, tag="warm_w")
            nc.vector.memset(warm_w[:], 0.0)
            warm_x = consts.tile([128, 256], dt.float16, tag="warm_x")
            nc.vector.memset(warm_x[:], 0.0)

            zpools = (pz0, pz1)

            xts = {}   # group -> xt tile
            ots = {}   # group -> output accumulation tile
            encs = {}  # (supertile, half) -> e tile
            h1ss = {}  # supertile -> h1s tile
            h2ss = {}  # supertile -> h2s tile

            # Initial HAM warm-up: ~5us of back-to-back full-array matmuls.
            # Masked (tile_position) matmuls never register as PE activity, so
            # without this the whole kernel runs at the cold 1.2 GHz clock.
            wz = pz0.tile([128, 2 * B], dt.float32, tag="z0", name="warmburst")
            for _ in range(24):
                nc.tensor.matmul(wz[:, 0:256], lhsT=warm_w[:], rhs=warm_x[:],
                                 start=True, stop=True)

            def ensure_group(g):
                if g in xts or g >= n_groups:
                    return
                g0 = g * GROUP_RAYS
                xt = xpool.tile([128, GROUP_RAYS], dt.float16, tag="xt",
                                name=f"xt{g}")
                nc.sync.dma_start(out=xt[0:KROWS, :],
                                  in_=xp[0:KROWS, g0 : g0 + GROUP_RAYS])
                nc.sync.dma_start(out=xt[64 : 64 + KROWS, :],
                                  in_=xp[KROWS : 2 * KROWS, g0 : g0 + GROUP_RAYS])
                xts[g] = xt
                ots[g] = outp.tile([128, B * G], dt.float16, tag="ot",
                                   name=f"ot{g}")

            def emit_expand_exp(t, half):
                """One 512-ray half: 2 concurrent expand MMs + 1 Exp op.

                z layout [128, 1024]: cols 0:512 = enc rows 0:128 (hi matmul,
                PE rows 0:37), cols 512:1024 = enc rows 128:224 + 32 pad rows
                (lo matmul, PE rows 64:101) -- disjoint row groups, so the two
                matmuls stream concurrently.
                """
                g, j = divmod(t, G)
                xt = xts[g]
                c0 = j * SUPER + half * B
                cols = slice(c0, c0 + B)
                z = zpools[half].tile([128, 2 * B], dt.float32,
                                      tag=f"z{half}", name=f"z{t}_{half}")
                e = encp.tile([128, 2 * B], dt.float16, tag="e",
                              name=f"e{t}_{half}")
                nc.tensor.matmul(
                    z[:, 0:B], lhsT=lw2_t[0:KROWS, 0:128],
                    rhs=xt[0:KROWS, cols],
                    start=True, stop=True, tile_position=(0, 0),
                )
                nc.tensor.matmul(
                    z[:, B : 2 * B], lhsT=lw2_t[64 : 64 + KROWS, 128:256],
                    rhs=xt[64 : 64 + KROWS, cols],
                    start=True, stop=True, tile_position=(64, 0),
                )
                nc.scalar.activation(e[:], z[:], Act.Exp)
                encs[(t, half)] = e

            def emit_l1(i):
                eA = encs.pop((i, 0))
                eB = encs.pop((i, 1))
                h1 = ph.tile([128, B], dt.float32, tag="hh", name=f"h1_{i}")
                # Unmasked dummy matmul: keeps the PE HAM warm (masked MMs
                # don't count as activity). Output garbage lands in h1 cols
                # 0:64 and is fully overwritten by the start=True MMs below.
                nc.tensor.matmul(h1[:, 0:HID], lhsT=warm_w[:],
                                 rhs=warm_x[:, 0:HID], start=True, stop=True)
                nc.tensor.matmul(h1[0:64, :], lhsT=w1a_t[:], rhs=eA[:, 0:B],
                                 start=True, stop=False, tile_position=(0, 0))
                nc.tensor.matmul(h1[64:128, :], lhsT=w1a_t[:], rhs=eB[:, 0:B],
                                 start=True, stop=False, tile_position=(0, 64))
                nc.tensor.matmul(h1[0:64, :], lhsT=w1b_t[:],
                                 rhs=eA[0 : ENC - 128, B : 2 * B],
                                 start=False, stop=True, tile_position=(0, 0))
                nc.tensor.matmul(h1[64:128, :], lhsT=w1b_t[:],
                                 rhs=eB[0 : ENC - 128, B : 2 * B],
                                 start=False, stop=True, tile_position=(0, 64))
                h1s = hp.tile([128, B], dt.float16, tag="h1s", name=f"h1s{i}")
                nc.vector.tensor_scalar(h1s[:], h1[:], b1s_t[:], 0.0,
                                        Alu.add, Alu.max)
                h1ss[i] = h1s

            def emit_l2(i):
                h1s = h1ss.pop(i)
                h2 = ph.tile([128, B], dt.float32, tag="hh", name=f"h2_{i}")
                nc.tensor.matmul(h2[0:64, :], lhsT=w2s_t[0:64, :],
                                 rhs=h1s[0:64, :],
                                 start=True, stop=True, tile_position=(0, 0))
                nc.tensor.matmul(h2[64:128, :], lhsT=w2s_t[64:128, :],
                                 rhs=h1s[64:128, :],
                                 start=True, stop=True, tile_position=(64, 64))
                h2s = hp.tile([128, B], dt.float16, tag="h2s", name=f"h2s{i}")
                nc.vector.tensor_scalar(h2s[:], h2[:], b2s_t[:], 0.0,
                                        Alu.add, Alu.max)
                h2ss[i] = h2s

            def emit_l3(i):
                g, j = divmod(i, G)
                h2s = h2ss.pop(i)
                # Flipped quadrants: L3 occupies the (0,64)/(64,0) quadrants
                # so it runs concurrently with L2 of a later supertile, which
                # uses (0,0)/(64,64). Output rows are therefore [B-rays; A-rays].
                op = ph.tile([128, B], dt.float32, tag="hh", name=f"op{i}")
                nc.tensor.matmul(op[64:128, :], lhsT=w3s_t[0:64, :],
                                 rhs=h2s[0:64, :],
                                 start=True, stop=True, tile_position=(0, 64))
                nc.tensor.matmul(op[0:64, :], lhsT=w3s_t[64:128, :],
                                 rhs=h2s[64:128, :],
                                 start=True, stop=True, tile_position=(64, 0))
                nc.vector.tensor_scalar_add(ots[g][:, j * B : (j + 1) * B],
                                            op[:], b3s_t[:])
                if j == G - 1:
                    half = B * G
                    nc.sync.dma_start(out=yt[:, g * half : (g + 1) * half],
                                      in_=ots[g][:])
                    del xts[g], ots[g]

            # Pipeline: expand+exp(t) | L1/L2(t-2) | L3(t-3)
            for t in range(n_super + 3):
                if t < n_super:
                    ensure_group(t // G)
                    emit_expand_exp(t, 0)
                    emit_expand_exp(t, 1)
                if 0 <= t - 2 < n_super:
                    emit_l1(t - 2)
                    emit_l2(t - 2)
                if 0 <= t - 3 < n_super:
                    emit_l3(t - 3)

    nc.finalize()
    return nc


def _get_nc():
    key = (NC_RAYS, N_GROUPS)
    if key not in _BUILD_CACHE:
        _BUILD_CACHE[key] = _build_bass(*key)
    return _BUILD_CACHE[key]


def _f16_hilo(x64):
    """Exact hi/lo split: x ~= hi + lo with hi, lo fp16 (inputs are fp64)."""
    hi = x64.astype(np.float16)
    lo = (x64 - hi.astype(np.float64)).astype(np.float16)
    return hi, lo


def _pack_weights(W1, b1, W2, b2, W3, b3, centers):
    c = centers.astype(np.float64)  # [32]
    inv2s2 = 0.5 / (SIGMA * SIGMA)  # 512
    # Per enc-row dj (d = dj//32, j = dj%32):
    #   z = -inv2s2*x_d^2 + (2*inv2s2*c_j)*x_d - inv2s2*c_j^2
    wx = 2.0 * inv2s2 * c  # [32] coefficient on x
    wq = -inv2s2  # coefficient on q = x^2 (exact in fp16)
    wu = -inv2s2 * c * c  # [32] coefficient on 1

    wx_hi = wx.astype(np.float16)
    wx_lo = (wx - wx_hi.astype(np.float64)).astype(np.float16)
    wu_hi = wu.astype(np.float16)
    wu_lo = (wu - wu_hi.astype(np.float64)).astype(np.float16)

    L = np.zeros((KROWS, ENC), np.float16)
    for d in range(IN_DIMS):
        cols = slice(d * BINS, (d + 1) * BINS)
        L[d, cols] = wx_hi
        L[7 + d, cols] = wx_lo
        L[14 + d, cols] = wx_hi
        L[21 + d, cols] = np.float16(wq)
        L[28 + d, cols] = np.float16(wq)
    L[35, :] = np.tile(wu_hi, IN_DIMS)
    L[36, :] = np.tile(wu_lo, IN_DIMS)

    # lw [37, 256]: cols 0:224 = enc weights; cols 224:256 repeat enc cols
    # 128:160 so the lo expand matmul writes all 128 PSUM partitions with
    # benign values (exp'd and ignored by L1).
    lw = np.zeros((KROWS, 256), np.float16)
    lw[:, 0:ENC] = L
    lw[:, ENC:256] = L[:, 128 : 128 + (256 - ENC)]

    w1 = W1.astype(np.float16)
    packs = {
        "lw": lw,
        "w1a": np.ascontiguousarray(w1[0:128]),
        "w1b": np.ascontiguousarray(w1[128:ENC]),
        "w2s": np.concatenate([W2, W2], 0).astype(np.float16),
        "w3s": np.concatenate([W3, W3], 0).astype(np.float16),
        "b1s": np.concatenate([b1, b1], 0).astype(np.float32).reshape(128, 1),
        "b2s": np.concatenate([b2, b2], 0).astype(np.float32).reshape(128, 1),
        "b3s": np.concatenate([b3, b3], 0).astype(np.float32).reshape(128, 1),
    }
    return packs


def _pack_inputs(pos01, wi01, rough01):
    x = np.concatenate(
        [np.asarray(pos01), np.asarray(wi01), np.asarray(rough01)], axis=1
    ).astype(np.float32)
    np.clip(x, 0.0, 1.0, out=x)
    x64 = x.astype(np.float64)
    q64 = x64 * x64
    x_hi, x_lo = _f16_hilo(x64)
    q_hi, q_lo = _f16_hilo(q64)
    ones = np.ones((x.shape[0], 2), np.float16)
    P = np.concatenate([x_hi, x_hi, x_lo, q_hi, q_lo, ones], axis=1)  # [N, 37]
    Pt = np.ascontiguousarray(P.T)  # [37, N] fp16
    xp = np.empty((2 * KROWS, x.shape[0]), np.float16)
    xp[0:KROWS] = Pt
    xp[KROWS : 2 * KROWS] = Pt
    return xp


def kernel(pos01, wi01, rough01, W1, b1, W2, b2, W3, b3, centers):
    global LAST_RESULTS
    import os

    from concourse.bass_utils import run_bass_kernel_spmd

    nc = _get_nc()

    xp = _pack_inputs(pos01, wi01, rough01)
    wpacks = _pack_weights(
        np.asarray(W1), np.asarray(b1), np.asarray(W2), np.asarray(b2),
        np.asarray(W3), np.asarray(b3), np.asarray(centers),
    )

    in_maps = []
    for c in range(N_CORES):
        m = dict(wpacks)
        m["xp"] = np.ascontiguousarray(xp[:, c * NC_RAYS : (c + 1) * NC_RAYS])
        in_maps.append(m)

    trace = bool(int(os.environ.get("KERNEL_TRACE", "0")))
    res = run_bass_kernel_spmd(nc, in_maps, list(range(N_CORES)), trace=trace)
    LAST_RESULTS = res

    out = np.empty((N_TOTAL, OUT), np.float32)
    for c in range(N_CORES):
        yt = res.results[c]["yt"]  # [128, NC_RAYS // 2] fp16
        arr = yt.reshape(128, N_GROUPS, G, B)
        # L3's flipped quadrants put A-half rays on rows 64:128, B on 0:64
        a = arr[OUT:128].transpose(1, 2, 3, 0)  # [g, j, r, 64]
        b = arr[0:OUT].transpose(1, 2, 3, 0)
        stacked = np.stack([a, b], axis=2)  # [g, j, 2, 512, 64]
        out[c * NC_RAYS : (c + 1) * NC_RAYS] = (
            stacked.reshape(NC_RAYS, OUT).astype(np.float32)
        )
    return out


# revision 11
# speedup vs baseline: 1.8345x; 1.8345x over previous
"""Trainium2 Bass kernel for a OneBlob-encoded 3-layer MLP (ConditioningNetwork).

Math:  x = clip(concat(pos01, wi01, rough01), 0, 1)          [N, 7]
       enc[n, d*32+j] = exp(-0.5 ((x[n,d]-c[j]) / sigma)^2)  [N, 224], sigma = 1/32
       y = relu(relu(enc@W1+b1)@W2+b2)@W3+b3                 [N, 64]

Strategy (pure data parallel over 8 cores, weights replicated):
  - The Gaussian exponent z = -(x-c)^2/(2 sigma^2) is affine in (x, x^2), so it is
    computed on the PE as one small matmul ("expand"):  z = L^T @ [x; x^2; 1].
    The quadratic has catastrophic cancellation near x == c, so x and x^2 are fed
    as exact fp16 hi+lo pairs and L carries hi/lo weight rows; all products are
    exact in fp16 (accumulated fp32 in PSUM) giving |dz| ~ 1e-3.
  - enc = Exp(z) on the scalar engine (ACT) straight out of PSUM, fp16 into SBUF.
  - Per 512-ray half, z is packed [128, 1024]: cols 0:512 = enc rows 0:128,
    cols 512:1024 = enc rows 128:224 (+ 32 benign pad rows so all 128 PSUM
    partitions are written). One ACT Exp op covers the whole half -> no idle
    ACT lanes and half-granular z double-buffering (2 tiles x 2 banks).
  - 3 MLP matmuls in fp16 (fp32 PSUM). A/B = consecutive 512-ray halves of a
    1024-ray supertile: L1 col-packed (psum partitions 0:64 / 64:128), L2/L3
    quadrant-packed (disjoint row+col groups -> concurrent on the PE).
    Bias+ReLU / bias+cast are single DVE tensor_scalar ops per [128, 512] tile.
  - The PE's HAM clock gate never warms on row/col-masked (tile_position)
    matmuls -- measured: they run at 1.2 GHz forever, but do run at 2.4 GHz
    once warm. So the kernel issues an unmasked full-array warm-up burst at
    start and one tiny unmasked dummy matmul per supertile (output is
    overwritten by L1's start accumulation) to keep the HAM at K=8/8.
  - Output is produced feature-major, packed [128, Nc/2] fp16 (per supertile:
    A-half rays on partitions 64:128, B-half on 0:64 -- L3 quadrant flip), and
    unpacked/transposed/cast on the host.

Input row packing (fp16, 37 rows, present twice in the [128, N] xp tensor:
rows 0:37 for the hi expand matmul, rows 64:101 for the lo one so the two
run concurrently on disjoint PE row groups; other rows dead -- the full
128-partition load is what spreads the DMA across all 16 SDMA engines):
  rows  0: 7  x_hi          (weights: w_hi,  w = c/sigma^2 per enc row)
  rows  7:14  x_hi (dup)    (weights: w_lo = w - fp16(w))
  rows 14:21  x_lo          (weights: w_hi)
  rows 21:28  q_hi, q = x^2 (weights: -1/(2 sigma^2), exact in fp16)
  rows 28:35  q_lo          (weights: -1/(2 sigma^2))
  row  35     ones          (weights: u_hi, u = -c^2/(2 sigma^2))
  row  36     ones          (weights: u_lo)
"""

import sys

import numpy as np

if "/opt/trn_rl_repo" not in sys.path:
    sys.path.insert(0, "/opt/trn_rl_repo")

N_CORES = 8
N_TOTAL = 1048576
NC_RAYS = N_TOTAL // N_CORES  # 131072 rays per core
BINS = 32
HID = 64
OUT = 64
IN_DIMS = 7
ENC = IN_DIMS * BINS  # 224
SIGMA = 1.0 / BINS

KROWS = 37  # packed input rows (see module docstring)
B = 512  # rays per matmul (one fp32 PSUM bank)
SUPER = 2 * B  # rays per supertile (A/B halves)
G = 8  # supertiles per DMA group
GROUP_RAYS = SUPER * G  # 8192
N_GROUPS = NC_RAYS // GROUP_RAYS  # 16

# Set by the last kernel() call so a test harness can read profile/exec time.
LAST_RESULTS = None

_BUILD_CACHE = {}


def _build_bass(nc_rays, n_groups):
    import concourse.tile as tile
    from concourse import bacc, mybir

    dt = mybir.dt
    Act = mybir.ActivationFunctionType
    Alu = mybir.AluOpType

    nc = bacc.Bacc("TRN2", target_bir_lowering=False, debug=False)

    n_super = n_groups * G

    # xp rows: 0:37 packed operand P, 64:101 duplicate of P (so hi/lo expand
    # matmuls stream from disjoint PE row groups and run concurrently); the
    # remaining rows are dead, padding the load to all 128 partitions so the
    # input DMA spreads across all 16 SDMA engines (non-128-partition loads
    # measured ~10us per 600KB -- they land on a fraction of the engines).
    xp = nc.dram_tensor("xp", [128, nc_rays], dt.float16, kind="ExternalInput")
    # lw [128, 256]: col block 0:128 = hi expand lhsT (rows 0:37 = packed
    # weights, rest zero), col block 128:256 = lo expand lhsT (rows 64:101 =
    # packed weights, rest zero). Full-height lhsT -> UNMASKED matmuls, which
    # are the only kind the PE's HAM clock gate counts as activity.
    lw = nc.dram_tensor("lw", [128, 256], dt.float16, kind="ExternalInput")
    w1a = nc.dram_tensor("w1a", [128, HID], dt.float16, kind="ExternalInput")
    w1b = nc.dram_tensor("w1b", [ENC - 128, HID], dt.float16, kind="ExternalInput")
    w2s = nc.dram_tensor("w2s", [128, HID], dt.float16, kind="ExternalInput")
    w3s = nc.dram_tensor("w3s", [128, OUT], dt.float16, kind="ExternalInput")
    b1s = nc.dram_tensor("b1s", [128, 1], dt.float32, kind="ExternalInput")
    b2s = nc.dram_tensor("b2s", [128, 1], dt.float32, kind="ExternalInput")
    b3s = nc.dram_tensor("b3s", [128, 1], dt.float32, kind="ExternalInput")
    # Output, packed fp16: per 512-col supertile block, rows 64:128 = A-half
    # rays, rows 0:64 = B-half rays (L3's flipped quadrants).
    yt = nc.dram_tensor("yt", [128, nc_rays // 2], dt.float16,
                        kind="ExternalOutput")

    with tile.TileContext(nc) as tc:
        with (
            tc.tile_pool(name="consts", bufs=1) as consts,
            tc.tile_pool(name="xpool", bufs=3) as xpool,
            tc.tile_pool(name="encp", bufs=4) as encp,
            tc.tile_pool(name="hp", bufs=3) as hp,
            tc.tile_pool(name="outp", bufs=2) as outp,
            tc.tile_pool(name="pz0", bufs=1, space="PSUM") as pz0,
            tc.tile_pool(name="pz1", bufs=1, space="PSUM") as pz1,
            tc.tile_pool(name="ph", bufs=4, space="PSUM") as ph,
        ):
            lw2_t = consts.tile([128, 256], dt.float16, tag="lw2_t")
            nc.sync.dma_start(out=lw2_t[:], in_=lw[:])
            w1a_t = consts.tile([128, HID], dt.float16, tag="w1a_t")
            nc.sync.dma_start(out=w1a_t[:], in_=w1a[:])
            w1b_t = consts.tile([ENC - 128, HID], dt.float16, tag="w1b_t")
            nc.sync.dma_start(out=w1b_t[:], in_=w1b[:])
            w2s_t = consts.tile([128, HID], dt.float16, tag="w2s_t")
            nc.sync.dma_start(out=w2s_t[:], in_=w2s[:])
            w3s_t = consts.tile([128, OUT], dt.float16, tag="w3s_t")
            nc.sync.dma_start(out=w3s_t[:], in_=w3s[:])
            b1s_t = consts.tile([128, 1], dt.float32, tag="b1s_t")
            nc.sync.dma_start(out=b1s_t[:], in_=b1s[:])
            b2s_t = consts.tile([128, 1], dt.float32, tag="b2s_t")
            nc.sync.dma_start(out=b2s_t[:], in_=b2s[:])
            b3s_t = consts.tile([128, 1], dt.float32, tag="b3s_t")
            nc.sync.dma_start(out=b3s_t[:], in_=b3s[:])

            # Zero tiles driving the unmasked HAM warm-up matmuls.
            warm_w = consts.tile([128, 128], dt.float16, tag="warm_w")
            nc.vector.memset(warm_w[:], 0.0)
            warm_x = consts.tile([128, 256], dt.float16, tag="warm_x")
            nc.vector.memset(warm_x[:], 0.0)

            zpools = (pz0, pz1)

            xts = {}   # group -> xt tile
            ots = {}   # group -> output accumulation tile
            encs = {}  # (supertile, half) -> e tile
            h1ss = {}  # supertile -> h1s tile
            h2ss = {}  # supertile -> h2s tile

            # Initial HAM warm-up: ~5us of back-to-back full-array matmuls.
            # Masked (tile_position) matmuls never register as PE activity, so
            # without this the whole kernel runs at the cold 1.2 GHz clock.
            wz = pz0.tile([128, 2 * B], dt.float32, tag="z0", name="warmburst")
            for _ in range(24):
                nc.tensor.matmul(wz[:, 0:256], lhsT=warm_w[:], rhs=warm_x[:],
                                 start=True, stop=True)

            def ensure_group(g):
                if g in xts or g >= n_groups:
                    return
                g0 = g * GROUP_RAYS
                xt = xpool.tile([128, GROUP_RAYS], dt.float16, tag="xt",
                                name=f"xt{g}")
                nc.sync.dma_start(out=xt[:], in_=xp[:, g0 : g0 + GROUP_RAYS])
                xts[g] = xt
                ots[g] = outp.tile([128, B * G], dt.float16, tag="ot",
                                   name=f"ot{g}")

            def emit_expand_exp(t, half):
                """One 512-ray half: 2 full-array expand MMs + 1 Exp op.

                z layout [128, 1024]: cols 0:512 = enc rows 0:128 (hi lhsT,
                nonzero weight rows 0:37), cols 512:1024 = enc rows 128:224 +
                32 pad rows (lo lhsT, nonzero rows 64:101). Both matmuls use
                full [128, 128] lhsT with zero padding and the full-height xt
                (dead rows are zeros), so they're UNMASKED: this is what keeps
                the PE's HAM clock gate at 2.4 GHz -- masked (tile_position)
                matmuls never register as PE activity, and a cold PE runs
                everything at 1.2 GHz.
                """
                g, j = divmod(t, G)
                xt = xts[g]
                c0 = j * SUPER + half * B
                cols = slice(c0, c0 + B)
                z = zpools[half].tile([128, 2 * B], dt.float32,
                                      tag=f"z{half}", name=f"z{t}_{half}")
                e = encp.tile([128, 2 * B], dt.float16, tag="e",
                              name=f"e{t}_{half}")
                nc.tensor.matmul(
                    z[:, 0:B], lhsT=lw2_t[:, 0:128], rhs=xt[:, cols],
                    start=True, stop=True,
                )
                nc.tensor.matmul(
                    z[:, B : 2 * B], lhsT=lw2_t[:, 128:256], rhs=xt[:, cols],
                    start=True, stop=True,
                )
                nc.scalar.activation(e[:], z[:], Act.Exp)
                encs[(t, half)] = e

            def emit_l1(i):
                eA = encs.pop((i, 0))
                eB = encs.pop((i, 1))
                h1 = ph.tile([128, B], dt.float32, tag="hh", name=f"h1_{i}")
                nc.tensor.matmul(h1[0:64, :], lhsT=w1a_t[:], rhs=eA[:, 0:B],
                                 start=True, stop=False, tile_position=(0, 0))
                nc.tensor.matmul(h1[64:128, :], lhsT=w1a_t[:], rhs=eB[:, 0:B],
                                 start=True, stop=False, tile_position=(0, 64))
                nc.tensor.matmul(h1[0:64, :], lhsT=w1b_t[:],
                                 rhs=eA[0 : ENC - 128, B : 2 * B],
                                 start=False, stop=True, tile_position=(0, 0))
                nc.tensor.matmul(h1[64:128, :], lhsT=w1b_t[:],
                                 rhs=eB[0 : ENC - 128, B : 2 * B],
                                 start=False, stop=True, tile_position=(0, 64))
                h1s = hp.tile([128, B], dt.float16, tag="h1s", name=f"h1s{i}")
                nc.vector.tensor_scalar(h1s[:], h1[:], b1s_t[:], 0.0,
                                        Alu.add, Alu.max)
                h1ss[i] = h1s

            def emit_l2(i):
                h1s = h1ss.pop(i)
                h2 = ph.tile([128, B], dt.float32, tag="hh", name=f"h2_{i}")
                nc.tensor.matmul(h2[0:64, :], lhsT=w2s_t[0:64, :],
                                 rhs=h1s[0:64, :],
                                 start=True, stop=True, tile_position=(0, 0))
                nc.tensor.matmul(h2[64:128, :], lhsT=w2s_t[64:128, :],
                                 rhs=h1s[64:128, :],
                                 start=True, stop=True, tile_position=(64, 64))
                h2s = hp.tile([128, B], dt.float16, tag="h2s", name=f"h2s{i}")
                nc.vector.tensor_scalar(h2s[:], h2[:], b2s_t[:], 0.0,
                                        Alu.add, Alu.max)
                h2ss[i] = h2s

            def emit_l3(i):
                g, j = divmod(i, G)
                h2s = h2ss.pop(i)
                # Flipped quadrants: L3 occupies the (0,64)/(64,0) quadrants
                # so it runs concurrently with L2 of a later supertile, which
                # uses (0,0)/(64,64). Output rows are therefore [B-rays; A-rays].
                op = ph.tile([128, B], dt.float32, tag="hh", name=f"op{i}")
                nc.tensor.matmul(op[64:128, :], lhsT=w3s_t[0:64, :],
                                 rhs=h2s[0:64, :],
                                 start=True, stop=True, tile_position=(0, 64))
                nc.tensor.matmul(op[0:64, :], lhsT=w3s_t[64:128, :],
                                 rhs=h2s[64:128, :],
                                 start=True, stop=True, tile_position=(64, 0))
                nc.vector.tensor_scalar_add(ots[g][:, j * B : (j + 1) * B],
                                            op[:], b3s_t[:])
                if j == G - 1:
                    half = B * G
                    nc.sync.dma_start(out=yt[:, g * half : (g + 1) * half],
                                      in_=ots[g][:])
                    del xts[g], ots[g]

            # Pipeline: expand+exp(t) | L1/L2(t-2) | L3(t-3)
            for t in range(n_super + 3):
                if t < n_super:
                    ensure_group(t // G)
                    emit_expand_exp(t, 0)
                    emit_expand_exp(t, 1)
                if 0 <= t - 2 < n_super:
                    emit_l1(t - 2)
                    emit_l2(t - 2)
                if 0 <= t - 3 < n_super:
                    emit_l3(t - 3)

    nc.finalize()
    return nc


def _get_nc():
    key = (NC_RAYS, N_GROUPS)
    if key not in _BUILD_CACHE:
        _BUILD_CACHE[key] = _build_bass(*key)
    return _BUILD_CACHE[key]


def _f16_hilo(x64):
    """Exact hi/lo split: x ~= hi + lo with hi, lo fp16 (inputs are fp64)."""
    hi = x64.astype(np.float16)
    lo = (x64 - hi.astype(np.float64)).astype(np.float16)
    return hi, lo


def _pack_weights(W1, b1, W2, b2, W3, b3, centers):
    c = centers.astype(np.float64)  # [32]
    inv2s2 = 0.5 / (SIGMA * SIGMA)  # 512
    # Per enc-row dj (d = dj//32, j = dj%32):
    #   z = -inv2s2*x_d^2 + (2*inv2s2*c_j)*x_d - inv2s2*c_j^2
    wx = 2.0 * inv2s2 * c  # [32] coefficient on x
    wq = -inv2s2  # coefficient on q = x^2 (exact in fp16)
    wu = -inv2s2 * c * c  # [32] coefficient on 1

    wx_hi = wx.astype(np.float16)
    wx_lo = (wx - wx_hi.astype(np.float64)).astype(np.float16)
    wu_hi = wu.astype(np.float16)
    wu_lo = (wu - wu_hi.astype(np.float64)).astype(np.float16)

    L = np.zeros((KROWS, ENC), np.float16)
    for d in range(IN_DIMS):
        cols = slice(d * BINS, (d + 1) * BINS)
        L[d, cols] = wx_hi
        L[7 + d, cols] = wx_lo
        L[14 + d, cols] = wx_hi
        L[21 + d, cols] = np.float16(wq)
        L[28 + d, cols] = np.float16(wq)
    L[35, :] = np.tile(wu_hi, IN_DIMS)
    L[36, :] = np.tile(wu_lo, IN_DIMS)

    # lw [128, 256]: col block 0:128 = hi lhsT (enc rows 0:128; weight rows
    # 0:37, rest zero), col block 128:256 = lo lhsT (enc rows 128:224 + 32
    # benign pad rows repeating enc 128:160; weight rows 64:101 -- the xp dup
    # block -- rest zero). Full-height zero-padded lhsT keeps the expand
    # matmuls unmasked (see _build_bass docstring).
    Lpad = np.concatenate([L[:, 128:ENC], L[:, 128 : 128 + (256 - ENC)]], 1)
    lw = np.zeros((128, 256), np.float16)
    lw[0:KROWS, 0:128] = L[:, 0:128]
    lw[64 : 64 + KROWS, 128:256] = Lpad

    w1 = W1.astype(np.float16)
    packs = {
        "lw": lw,
        "w1a": np.ascontiguousarray(w1[0:128]),
        "w1b": np.ascontiguousarray(w1[128:ENC]),
        "w2s": np.concatenate([W2, W2], 0).astype(np.float16),
        "w3s": np.concatenate([W3, W3], 0).astype(np.float16),
        "b1s": np.concatenate([b1, b1], 0).astype(np.float32).reshape(128, 1),
        "b2s": np.concatenate([b2, b2], 0).astype(np.float32).reshape(128, 1),
        "b3s": np.concatenate([b3, b3], 0).astype(np.float32).reshape(128, 1),
    }
    return packs


def _pack_inputs(pos01, wi01, rough01):
    x = np.concatenate(
        [np.asarray(pos01), np.asarray(wi01), np.asarray(rough01)], axis=1
    ).astype(np.float32)
    np.clip(x, 0.0, 1.0, out=x)
    x64 = x.astype(np.float64)
    q64 = x64 * x64
    x_hi, x_lo = _f16_hilo(x64)
    q_hi, q_lo = _f16_hilo(q64)
    ones = np.ones((x.shape[0], 2), np.float16)
    P = np.concatenate([x_hi, x_hi, x_lo, q_hi, q_lo, ones], axis=1)  # [N, 37]
    Pt = np.ascontiguousarray(P.T)  # [37, N] fp16
    xp = np.zeros((128, x.shape[0]), np.float16)
    xp[0:KROWS] = Pt
    xp[64 : 64 + KROWS] = Pt
    return xp


def kernel(pos01, wi01, rough01, W1, b1, W2, b2, W3, b3, centers):
    global LAST_RESULTS
    import os

    from concourse.bass_utils import run_bass_kernel_spmd

    nc = _get_nc()

    xp = _pack_inputs(pos01, wi01, rough01)
    wpacks = _pack_weights(
        np.asarray(W1), np.asarray(b1), np.asarray(W2), np.asarray(b2),
        np.asarray(W3), np.asarray(b3), np.asarray(centers),
    )

    in_maps = []
    for c in range(N_CORES):
        m = dict(wpacks)
        m["xp"] = np.ascontiguousarray(xp[:, c * NC_RAYS : (c + 1) * NC_RAYS])
        in_maps.append(m)

    trace = bool(int(os.environ.get("KERNEL_TRACE", "0")))
    res = run_bass_kernel_spmd(nc, in_maps, list(range(N_CORES)), trace=trace)
    LAST_RESULTS = res

    out = np.empty((N_TOTAL, OUT), np.float32)
    for c in range(N_CORES):
        yt = res.results[c]["yt"]  # [128, NC_RAYS // 2] fp16
        arr = yt.reshape(128, N_GROUPS, G, B)
        # L3's flipped quadrants put A-half rays on rows 64:128, B on 0:64
        a = arr[OUT:128].transpose(1, 2, 3, 0)  # [g, j, r, 64]
        b = arr[0:OUT].transpose(1, 2, 3, 0)
        stacked = np.stack([a, b], axis=2)  # [g, j, 2, 512, 64]
        out[c * NC_RAYS : (c + 1) * NC_RAYS] = (
            stacked.reshape(NC_RAYS, OUT).astype(np.float32)
        )
    return out


# revision 13
# speedup vs baseline: 2.7229x; 1.4843x over previous
"""Trainium2 Bass kernel for a OneBlob-encoded 3-layer MLP (ConditioningNetwork).

Math:  x = clip(concat(pos01, wi01, rough01), 0, 1)          [N, 7]
       enc[n, d*32+j] = exp(-0.5 ((x[n,d]-c[j]) / sigma)^2)  [N, 224], sigma = 1/32
       y = relu(relu(enc@W1+b1)@W2+b2)@W3+b3                 [N, 64]

Strategy (pure data parallel over 8 cores, weights replicated):
  - The Gaussian exponent z = -(x-c)^2/(2 sigma^2) is affine in (x, x^2), so it is
    computed on the PE as one small matmul ("expand"):  z = L^T @ [x; x^2; 1].
    The quadratic has catastrophic cancellation near x == c, so x and x^2 are fed
    as exact fp16 hi+lo pairs and L carries hi/lo weight rows; all products are
    exact in fp16 (accumulated fp32 in PSUM) giving |dz| ~ 1e-3.
  - enc = Exp(z) on the scalar engine (ACT) straight out of PSUM, fp16 into SBUF.
  - Per 512-ray half, z is packed [128, 1024]: cols 0:512 = enc rows 0:128,
    cols 512:1024 = enc rows 128:224 (+ 32 benign pad rows so all 128 PSUM
    partitions are written). One ACT Exp op covers the whole half -> no idle
    ACT lanes and half-granular z double-buffering (2 tiles x 2 banks).
  - 3 MLP matmuls in fp16 (fp32 PSUM). A/B = consecutive 512-ray halves of a
    1024-ray supertile: L1 col-packed (psum partitions 0:64 / 64:128), L2/L3
    quadrant-packed (disjoint row+col groups -> concurrent on the PE).
    Bias+ReLU / bias+cast are single DVE tensor_scalar ops per [128, 512] tile.
  - The PE's HAM clock gate never warms on row/col-masked (tile_position)
    matmuls -- measured: they run at 1.2 GHz forever, but do run at 2.4 GHz
    once warm. So the kernel issues an unmasked full-array warm-up burst at
    start and one tiny unmasked dummy matmul per supertile (output is
    overwritten by L1's start accumulation) to keep the HAM at K=8/8.
  - Output is produced feature-major, packed [128, Nc/2] fp16 (per supertile:
    A-half rays on partitions 64:128, B-half on 0:64 -- L3 quadrant flip), and
    unpacked/transposed/cast on the host.

Input row packing (fp16, 37 rows, present twice in the [128, N] xp tensor:
rows 0:37 for the hi expand matmul, rows 64:101 for the lo one so the two
run concurrently on disjoint PE row groups; other rows dead -- the full
128-partition load is what spreads the DMA across all 16 SDMA engines):
  rows  0: 7  x_hi          (weights: w_hi,  w = c/sigma^2 per enc row)
  rows  7:14  x_hi (dup)    (weights: w_lo = w - fp16(w))
  rows 14:21  x_lo          (weights: w_hi)
  rows 21:28  q_hi, q = x^2 (weights: -1/(2 sigma^2), exact in fp16)
  rows 28:35  q_lo          (weights: -1/(2 sigma^2))
  row  35     ones          (weights: u_hi, u = -c^2/(2 sigma^2))
  row  36     ones          (weights: u_lo)
"""

import sys

import numpy as np

if "/opt/trn_rl_repo" not in sys.path:
    sys.path.insert(0, "/opt/trn_rl_repo")

N_CORES = 8
N_TOTAL = 1048576
NC_RAYS = N_TOTAL // N_CORES  # 131072 rays per core
BINS = 32
HID = 64
OUT = 64
IN_DIMS = 7
ENC = IN_DIMS * BINS  # 224
SIGMA = 1.0 / BINS

KROWS = 37  # packed input rows (see module docstring)
B = 512  # rays per matmul (one fp32 PSUM bank)
SUPER = 2 * B  # rays per supertile (A/B halves)
G = 8  # supertiles per DMA group
GROUP_RAYS = SUPER * G  # 8192
N_GROUPS = NC_RAYS // GROUP_RAYS  # 16

# Set by the last kernel() call so a test harness can read profile/exec time.
LAST_RESULTS = None

_BUILD_CACHE = {}


def _build_bass(nc_rays, n_groups):
    import concourse.tile as tile
    from concourse import bacc, mybir

    dt = mybir.dt
    Act = mybir.ActivationFunctionType
    Alu = mybir.AluOpType

    nc = bacc.Bacc("TRN2", target_bir_lowering=False, debug=False)

    n_super = n_groups * G

    # xp rows: 0:37 packed operand P, 64:101 duplicate of P (so hi/lo expand
    # matmuls stream from disjoint PE row groups and run concurrently); the
    # remaining rows are dead, padding the load to all 128 partitions so the
    # input DMA spreads across all 16 SDMA engines (non-128-partition loads
    # measured ~10us per 600KB -- they land on a fraction of the engines).
    xp = nc.dram_tensor("xp", [128, nc_rays], dt.float16, kind="ExternalInput")
    # lw [128, 256]: col block 0:128 = hi expand lhsT (rows 0:37 = packed
    # weights, rest zero), col block 128:256 = lo expand lhsT (rows 64:101 =
    # packed weights, rest zero). Full-height lhsT -> UNMASKED matmuls, which
    # are the only kind the PE's HAM clock gate counts as activity.
    lw = nc.dram_tensor("lw", [128, 256], dt.float16, kind="ExternalInput")
    w1a = nc.dram_tensor("w1a", [128, HID], dt.float16, kind="ExternalInput")
    w1b = nc.dram_tensor("w1b", [ENC - 128, HID], dt.float16, kind="ExternalInput")
    w2s = nc.dram_tensor("w2s", [128, HID], dt.float16, kind="ExternalInput")
    w3s = nc.dram_tensor("w3s", [128, OUT], dt.float16, kind="ExternalInput")
    b1s = nc.dram_tensor("b1s", [128, 1], dt.float32, kind="ExternalInput")
    b2s = nc.dram_tensor("b2s", [128, 1], dt.float32, kind="ExternalInput")
    b3s = nc.dram_tensor("b3s", [128, 1], dt.float32, kind="ExternalInput")
    # Output, packed fp16: per 512-col supertile block, rows 64:128 = A-half
    # rays, rows 0:64 = B-half rays (L3's flipped quadrants).
    yt = nc.dram_tensor("yt", [128, nc_rays // 2], dt.float16,
                        kind="ExternalOutput")

    with tile.TileContext(nc) as tc:
        with (
            tc.tile_pool(name="consts", bufs=1) as consts,
            tc.tile_pool(name="xpool", bufs=3) as xpool,
            tc.tile_pool(name="encp", bufs=8) as encp,
            tc.tile_pool(name="hp", bufs=4) as hp,
            tc.tile_pool(name="outp", bufs=2) as outp,
            tc.tile_pool(name="pz0", bufs=1, space="PSUM") as pz0,
            tc.tile_pool(name="pz1", bufs=1, space="PSUM") as pz1,
            tc.tile_pool(name="ph", bufs=4, space="PSUM") as ph,
        ):
            lw2_t = consts.tile([128, 256], dt.float16, tag="lw2_t")
            nc.sync.dma_start(out=lw2_t[:], in_=lw[:])
            w1a_t = consts.tile([128, HID], dt.float16, tag="w1a_t")
            nc.sync.dma_start(out=w1a_t[:], in_=w1a[:])
            w1b_t = consts.tile([ENC - 128, HID], dt.float16, tag="w1b_t")
            nc.sync.dma_start(out=w1b_t[:], in_=w1b[:])
            w2s_t = consts.tile([128, HID], dt.float16, tag="w2s_t")
            nc.sync.dma_start(out=w2s_t[:], in_=w2s[:])
            w3s_t = consts.tile([128, OUT], dt.float16, tag="w3s_t")
            nc.sync.dma_start(out=w3s_t[:], in_=w3s[:])
            b1s_t = consts.tile([128, 1], dt.float32, tag="b1s_t")
            nc.sync.dma_start(out=b1s_t[:], in_=b1s[:])
            b2s_t = consts.tile([128, 1], dt.float32, tag="b2s_t")
            nc.sync.dma_start(out=b2s_t[:], in_=b2s[:])
            b3s_t = consts.tile([128, 1], dt.float32, tag="b3s_t")
            nc.sync.dma_start(out=b3s_t[:], in_=b3s[:])

            # Zero tiles driving the unmasked HAM warm-up matmuls.
            warm_w = consts.tile([128, 128], dt.float16, tag="warm_w")
            nc.vector.memset(warm_w[:], 0.0)
            warm_x = consts.tile([128, 256], dt.float16, tag="warm_x")
            nc.vector.memset(warm_x[:], 0.0)

            zpools = (pz0, pz1)

            xts = {}   # group -> xt tile
            ots = {}   # group -> output accumulation tile
            encs = {}  # (supertile, half) -> e tile
            h1ss = {}  # supertile -> h1s tile
            h2ss = {}  # supertile -> h2s tile

            # Initial HAM warm-up: ~5us of back-to-back full-array matmuls.
            # Masked (tile_position) matmuls never register as PE activity, so
            # without this the whole kernel runs at the cold 1.2 GHz clock.
            wz = pz0.tile([128, 2 * B], dt.float32, tag="z0", name="warmburst")
            for _ in range(24):
                nc.tensor.matmul(wz[:, 0:256], lhsT=warm_w[:], rhs=warm_x[:],
                                 start=True, stop=True)

            def ensure_group(g):
                if g in xts or g >= n_groups:
                    return
                g0 = g * GROUP_RAYS
                xt = xpool.tile([128, GROUP_RAYS], dt.float16, tag="xt",
                                name=f"xt{g}")
                nc.sync.dma_start(out=xt[:], in_=xp[:, g0 : g0 + GROUP_RAYS])
                xts[g] = xt
                ots[g] = outp.tile([128, B * G], dt.float16, tag="ot",
                                   name=f"ot{g}")

            def emit_expand_exp(t, half):
                """One 512-ray half: 2 full-array expand MMs + 1 Exp op.

                z layout [128, 1024]: cols 0:512 = enc rows 0:128 (hi lhsT,
                nonzero weight rows 0:37), cols 512:1024 = enc rows 128:224 +
                32 pad rows (lo lhsT, nonzero rows 64:101). Both matmuls use
                full [128, 128] lhsT with zero padding and the full-height xt
                (dead rows are zeros), so they're UNMASKED: this is what keeps
                the PE's HAM clock gate at 2.4 GHz -- masked (tile_position)
                matmuls never register as PE activity, and a cold PE runs
                everything at 1.2 GHz.
                """
                g, j = divmod(t, G)
                xt = xts[g]
                c0 = j * SUPER + half * B
                cols = slice(c0, c0 + B)
                z = zpools[half].tile([128, 2 * B], dt.float32,
                                      tag=f"z{half}", name=f"z{t}_{half}")
                e = encp.tile([128, 2 * B], dt.float16, tag="e",
                              name=f"e{t}_{half}")
                nc.tensor.matmul(
                    z[:, 0:B], lhsT=lw2_t[:, 0:128], rhs=xt[:, cols],
                    start=True, stop=True,
                )
                nc.tensor.matmul(
                    z[:, B : 2 * B], lhsT=lw2_t[:, 128:256], rhs=xt[:, cols],
                    start=True, stop=True,
                )
                nc.scalar.activation(e[:], z[:], Act.Exp)
                encs[(t, half)] = e

            def emit_l1(i):
                eA = encs.pop((i, 0))
                eB = encs.pop((i, 1))
                h1 = ph.tile([128, B], dt.float32, tag="hh", name=f"h1_{i}")
                nc.tensor.matmul(h1[0:64, :], lhsT=w1a_t[:], rhs=eA[:, 0:B],
                                 start=True, stop=False, tile_position=(0, 0))
                nc.tensor.matmul(h1[64:128, :], lhsT=w1a_t[:], rhs=eB[:, 0:B],
                                 start=True, stop=False, tile_position=(0, 64))
                nc.tensor.matmul(h1[0:64, :], lhsT=w1b_t[:],
                                 rhs=eA[0 : ENC - 128, B : 2 * B],
                                 start=False, stop=True, tile_position=(0, 0))
                nc.tensor.matmul(h1[64:128, :], lhsT=w1b_t[:],
                                 rhs=eB[0 : ENC - 128, B : 2 * B],
                                 start=False, stop=True, tile_position=(0, 64))
                h1s = hp.tile([128, B], dt.float16, tag="h1s", name=f"h1s{i}")
                nc.vector.tensor_scalar(h1s[:], h1[:], b1s_t[:], 0.0,
                                        Alu.add, Alu.max)
                h1ss[i] = h1s

            def emit_l2(i):
                h1s = h1ss.pop(i)
                h2 = ph.tile([128, B], dt.float32, tag="hh", name=f"h2_{i}")
                nc.tensor.matmul(h2[0:64, :], lhsT=w2s_t[0:64, :],
                                 rhs=h1s[0:64, :],
                                 start=True, stop=True, tile_position=(0, 0))
                nc.tensor.matmul(h2[64:128, :], lhsT=w2s_t[64:128, :],
                                 rhs=h1s[64:128, :],
                                 start=True, stop=True, tile_position=(64, 64))
                h2s = hp.tile([128, B], dt.float16, tag="h2s", name=f"h2s{i}")
                nc.vector.tensor_scalar(h2s[:], h2[:], b2s_t[:], 0.0,
                                        Alu.add, Alu.max)
                h2ss[i] = h2s

            def emit_l3(i):
                g, j = divmod(i, G)
                h2s = h2ss.pop(i)
                # Flipped quadrants: L3 occupies the (0,64)/(64,0) quadrants
                # so it runs concurrently with L2 of a later supertile, which
                # uses (0,0)/(64,64). Output rows are therefore [B-rays; A-rays].
                op = ph.tile([128, B], dt.float32, tag="hh", name=f"op{i}")
                nc.tensor.matmul(op[64:128, :], lhsT=w3s_t[0:64, :],
                                 rhs=h2s[0:64, :],
                                 start=True, stop=True, tile_position=(0, 64))
                nc.tensor.matmul(op[0:64, :], lhsT=w3s_t[64:128, :],
                                 rhs=h2s[64:128, :],
                                 start=True, stop=True, tile_position=(64, 0))
                nc.vector.tensor_scalar_add(ots[g][:, j * B : (j + 1) * B],
                                            op[:], b3s_t[:])
                if j == G - 1:
                    half = B * G
                    nc.sync.dma_start(out=yt[:, g * half : (g + 1) * half],
                                      in_=ots[g][:])
                    del xts[g], ots[g]

            # Pipeline: expand+exp(t) | L1(t-2) | L2(t-3) | L3(t-4).
            # Each DVE bounce (h1s, h2s) gets a full supertile of slack
            # before its consumer matmul, so DVE latency stays off the PE's
            # critical path (with L1/L2 in the same stage the PE idled ~0.9us
            # per supertile waiting on h1s).
            for t in range(n_super + 4):
                if t < n_super:
                    ensure_group(t // G)
                    emit_expand_exp(t, 0)
                    emit_expand_exp(t, 1)
                if 0 <= t - 2 < n_super:
                    emit_l1(t - 2)
                if 0 <= t - 3 < n_super:
                    emit_l2(t - 3)
                if 0 <= t - 4 < n_super:
                    emit_l3(t - 4)

    nc.finalize()
    return nc


def _get_nc():
    key = (NC_RAYS, N_GROUPS)
    if key not in _BUILD_CACHE:
        _BUILD_CACHE[key] = _build_bass(*key)
    return _BUILD_CACHE[key]


def _f16_hilo(x64):
    """Exact hi/lo split: x ~= hi + lo with hi, lo fp16 (inputs are fp64)."""
    hi = x64.astype(np.float16)
    lo = (x64 - hi.astype(np.float64)).astype(np.float16)
    return hi, lo


def _pack_weights(W1, b1, W2, b2, W3, b3, centers):
    c = centers.astype(np.float64)  # [32]
    inv2s2 = 0.5 / (SIGMA * SIGMA)  # 512
    # Per enc-row dj (d = dj//32, j = dj%32):
    #   z = -inv2s2*x_d^2 + (2*inv2s2*c_j)*x_d - inv2s2*c_j^2
    wx = 2.0 * inv2s2 * c  # [32] coefficient on x
    wq = -inv2s2  # coefficient on q = x^2 (exact in fp16)
    wu = -inv2s2 * c * c  # [32] coefficient on 1

    wx_hi = wx.astype(np.float16)
    wx_lo = (wx - wx_hi.astype(np.float64)).astype(np.float16)
    wu_hi = wu.astype(np.float16)
    wu_lo = (wu - wu_hi.astype(np.float64)).astype(np.float16)

    L = np.zeros((KROWS, ENC), np.float16)
    for d in range(IN_DIMS):
        cols = slice(d * BINS, (d + 1) * BINS)
        L[d, cols] = wx_hi
        L[7 + d, cols] = wx_lo
        L[14 + d, cols] = wx_hi
        L[21 + d, cols] = np.float16(wq)
        L[28 + d, cols] = np.float16(wq)
    L[35, :] = np.tile(wu_hi, IN_DIMS)
    L[36, :] = np.tile(wu_lo, IN_DIMS)

    # lw [128, 256]: col block 0:128 = hi lhsT (enc rows 0:128; weight rows
    # 0:37, rest zero), col block 128:256 = lo lhsT (enc rows 128:224 + 32
    # benign pad rows repeating enc 128:160; weight rows 64:101 -- the xp dup
    # block -- rest zero). Full-height zero-padded lhsT keeps the expand
    # matmuls unmasked (see _build_bass docstring).
    Lpad = np.concatenate([L[:, 128:ENC], L[:, 128 : 128 + (256 - ENC)]], 1)
    lw = np.zeros((128, 256), np.float16)
    lw[0:KROWS, 0:128] = L[:, 0:128]
    lw[64 : 64 + KROWS, 128:256] = Lpad

    w1 = W1.astype(np.float16)
    packs = {
        "lw": lw,
        "w1a": np.ascontiguousarray(w1[0:128]),
        "w1b": np.ascontiguousarray(w1[128:ENC]),
        "w2s": np.concatenate([W2, W2], 0).astype(np.float16),
        "w3s": np.concatenate([W3, W3], 0).astype(np.float16),
        "b1s": np.concatenate([b1, b1], 0).astype(np.float32).reshape(128, 1),
        "b2s": np.concatenate([b2, b2], 0).astype(np.float32).reshape(128, 1),
        "b3s": np.concatenate([b3, b3], 0).astype(np.float32).reshape(128, 1),
    }
    return packs


def _pack_inputs(pos01, wi01, rough01):
    x = np.concatenate(
        [np.asarray(pos01), np.asarray(wi01), np.asarray(rough01)], axis=1
    ).astype(np.float32)
    np.clip(x, 0.0, 1.0, out=x)
    x64 = x.astype(np.float64)
    q64 = x64 * x64
    x_hi, x_lo = _f16_hilo(x64)
    q_hi, q_lo = _f16_hilo(q64)
    ones = np.ones((x.shape[0], 2), np.float16)
    P = np.concatenate([x_hi, x_hi, x_lo, q_hi, q_lo, ones], axis=1)  # [N, 37]
    Pt = np.ascontiguousarray(P.T)  # [37, N] fp16
    xp = np.zeros((128, x.shape[0]), np.float16)
    xp[0:KROWS] = Pt
    xp[64 : 64 + KROWS] = Pt
    return xp


def kernel(pos01, wi01, rough01, W1, b1, W2, b2, W3, b3, centers):
    global LAST_RESULTS
    import os

    from concourse.bass_utils import run_bass_kernel_spmd

    nc = _get_nc()

    xp = _pack_inputs(pos01, wi01, rough01)
    wpacks = _pack_weights(
        np.asarray(W1), np.asarray(b1), np.asarray(W2), np.asarray(b2),
        np.asarray(W3), np.asarray(b3), np.asarray(centers),
    )

    in_maps = []
    for c in range(N_CORES):
        m = dict(wpacks)
        m["xp"] = np.ascontiguousarray(xp[:, c * NC_RAYS : (c + 1) * NC_RAYS])
        in_maps.append(m)

    trace = bool(int(os.environ.get("KERNEL_TRACE", "0")))
    res = run_bass_kernel_spmd(nc, in_maps, list(range(N_CORES)), trace=trace)
    LAST_RESULTS = res

    out = np.empty((N_TOTAL, OUT), np.float32)
    for c in range(N_CORES):
        yt = res.results[c]["yt"]  # [128, NC_RAYS // 2] fp16
        arr = yt.reshape(128, N_GROUPS, G, B)
        # L3's flipped quadrants put A-half rays on rows 64:128, B on 0:64
        a = arr[OUT:128].transpose(1, 2, 3, 0)  # [g, j, r, 64]
        b = arr[0:OUT].transpose(1, 2, 3, 0)
        stacked = np.stack([a, b], axis=2)  # [g, j, 2, 512, 64]
        out[c * NC_RAYS : (c + 1) * NC_RAYS] = (
            stacked.reshape(NC_RAYS, OUT).astype(np.float32)
        )
    return out


# revision 15
# speedup vs baseline: 2.7346x; 1.0043x over previous
"""Trainium2 Bass kernel for a OneBlob-encoded 3-layer MLP (ConditioningNetwork).

Math:  x = clip(concat(pos01, wi01, rough01), 0, 1)          [N, 7]
       enc[n, d*32+j] = exp(-0.5 ((x[n,d]-c[j]) / sigma)^2)  [N, 224], sigma = 1/32
       y = relu(relu(enc@W1+b1)@W2+b2)@W3+b3                 [N, 64]

Strategy (pure data parallel over 8 cores, weights replicated):
  - The Gaussian exponent z = -(x-c)^2/(2 sigma^2) is affine in (x, x^2), so it is
    computed on the PE as one small matmul ("expand"):  z = L^T @ [x; x^2; 1].
    The quadratic has catastrophic cancellation near x == c, so x and x^2 are fed
    as exact fp16 hi+lo pairs and L carries hi/lo weight rows; all products are
    exact in fp16 (accumulated fp32 in PSUM) giving |dz| ~ 1e-3.
  - enc = Exp(z) on the scalar engine (ACT) straight out of PSUM, fp16 into SBUF.
  - Per 512-ray half, z is packed [128, 1024]: cols 0:512 = enc rows 0:128,
    cols 512:1024 = enc rows 128:224 (+ 32 benign pad rows so all 128 PSUM
    partitions are written). One ACT Exp op covers the whole half -> no idle
    ACT lanes and half-granular z double-buffering (2 tiles x 2 banks).
  - 3 MLP matmuls in fp16 (fp32 PSUM). A/B = consecutive 512-ray halves of a
    1024-ray supertile: L1 col-packed (psum partitions 0:64 / 64:128), L2/L3
    quadrant-packed (disjoint row+col groups -> concurrent on the PE).
    Bias+ReLU / bias+cast are single DVE tensor_scalar ops per [128, 512] tile.
  - The PE's HAM clock gate never warms on row/col-masked (tile_position)
    matmuls -- measured: they run at 1.2 GHz forever, but do run at 2.4 GHz
    once warm. So the kernel issues an unmasked full-array warm-up burst at
    start and one tiny unmasked dummy matmul per supertile (output is
    overwritten by L1's start accumulation) to keep the HAM at K=8/8.
  - Output is produced feature-major, packed [128, Nc/2] fp16 (per supertile:
    A-half rays on partitions 64:128, B-half on 0:64 -- L3 quadrant flip), and
    unpacked/transposed/cast on the host.

Input row packing (fp16, 37 rows, present twice in the [128, N] xp tensor:
rows 0:37 for the hi expand matmul, rows 64:101 for the lo one so the two
run concurrently on disjoint PE row groups; other rows dead -- the full
128-partition load is what spreads the DMA across all 16 SDMA engines):
  rows  0: 7  x_hi          (weights: w_hi,  w = c/sigma^2 per enc row)
  rows  7:14  x_hi (dup)    (weights: w_lo = w - fp16(w))
  rows 14:21  x_lo          (weights: w_hi)
  rows 21:28  q_hi, q = x^2 (weights: -1/(2 sigma^2), exact in fp16)
  rows 28:35  q_lo          (weights: -1/(2 sigma^2))
  row  35     ones          (weights: u_hi, u = -c^2/(2 sigma^2))
  row  36     ones          (weights: u_lo)
"""

import sys

import numpy as np

if "/opt/trn_rl_repo" not in sys.path:
    sys.path.insert(0, "/opt/trn_rl_repo")

N_CORES = 8
N_TOTAL = 1048576
NC_RAYS = N_TOTAL // N_CORES  # 131072 rays per core
BINS = 32
HID = 64
OUT = 64
IN_DIMS = 7
ENC = IN_DIMS * BINS  # 224
SIGMA = 1.0 / BINS

KROWS = 37  # packed input rows (see module docstring)
B = 512  # rays per matmul (one fp32 PSUM bank)
SUPER = 2 * B  # rays per supertile (A/B halves)
G = 8  # supertiles per DMA group
GROUP_RAYS = SUPER * G  # 8192
N_GROUPS = NC_RAYS // GROUP_RAYS  # 16

# Set by the last kernel() call so a test harness can read profile/exec time.
LAST_RESULTS = None

_BUILD_CACHE = {}


def _build_bass(nc_rays, n_groups):
    import concourse.tile as tile
    from concourse import bacc, mybir

    dt = mybir.dt
    Act = mybir.ActivationFunctionType
    Alu = mybir.AluOpType

    nc = bacc.Bacc("TRN2", target_bir_lowering=False, debug=False)

    n_super = n_groups * G

    # xp rows: 0:37 packed operand P, 64:101 duplicate of P (so hi/lo expand
    # matmuls stream from disjoint PE row groups and run concurrently); the
    # remaining rows are dead, padding the load to all 128 partitions so the
    # input DMA spreads across all 16 SDMA engines (non-128-partition loads
    # measured ~10us per 600KB -- they land on a fraction of the engines).
    xp = nc.dram_tensor("xp", [128, nc_rays], dt.float16, kind="ExternalInput")
    # lw [128, 256]: col block 0:128 = hi expand lhsT (rows 0:37 = packed
    # weights, rest zero), col block 128:256 = lo expand lhsT (rows 64:101 =
    # packed weights, rest zero). Full-height lhsT -> UNMASKED matmuls, which
    # are the only kind the PE's HAM clock gate counts as activity.
    lw = nc.dram_tensor("lw", [128, 256], dt.float16, kind="ExternalInput")
    w1a = nc.dram_tensor("w1a", [128, HID], dt.float16, kind="ExternalInput")
    w1b = nc.dram_tensor("w1b", [ENC - 128, HID], dt.float16, kind="ExternalInput")
    w2s = nc.dram_tensor("w2s", [128, HID], dt.float16, kind="ExternalInput")
    w3s = nc.dram_tensor("w3s", [128, OUT], dt.float16, kind="ExternalInput")
    b1s = nc.dram_tensor("b1s", [128, 1], dt.float32, kind="ExternalInput")
    b2s = nc.dram_tensor("b2s", [128, 1], dt.float32, kind="ExternalInput")
    b3s = nc.dram_tensor("b3s", [128, 1], dt.float32, kind="ExternalInput")
    # Output, packed fp16: per 512-col supertile block, rows 64:128 = A-half
    # rays, rows 0:64 = B-half rays (L3's flipped quadrants).
    yt = nc.dram_tensor("yt", [128, nc_rays // 2], dt.float16,
                        kind="ExternalOutput")

    with tile.TileContext(nc) as tc:
        with (
            tc.tile_pool(name="consts", bufs=1) as consts,
            tc.tile_pool(name="xpool", bufs=3) as xpool,
            tc.tile_pool(name="encp", bufs=8) as encp,
            tc.tile_pool(name="hp", bufs=4) as hp,
            tc.tile_pool(name="outp", bufs=3) as outp,
            tc.tile_pool(name="pz0", bufs=1, space="PSUM") as pz0,
            tc.tile_pool(name="pz1", bufs=1, space="PSUM") as pz1,
            tc.tile_pool(name="ph", bufs=4, space="PSUM") as ph,
        ):
            lw2_t = consts.tile([128, 256], dt.float16, tag="lw2_t")
            nc.sync.dma_start(out=lw2_t[:], in_=lw[:])
            w1a_t = consts.tile([128, HID], dt.float16, tag="w1a_t")
            nc.sync.dma_start(out=w1a_t[:], in_=w1a[:])
            w1b_t = consts.tile([ENC - 128, HID], dt.float16, tag="w1b_t")
            nc.sync.dma_start(out=w1b_t[:], in_=w1b[:])
            w2s_t = consts.tile([128, HID], dt.float16, tag="w2s_t")
            nc.sync.dma_start(out=w2s_t[:], in_=w2s[:])
            w3s_t = consts.tile([128, OUT], dt.float16, tag="w3s_t")
            nc.sync.dma_start(out=w3s_t[:], in_=w3s[:])
            b1s_t = consts.tile([128, 1], dt.float32, tag="b1s_t")
            nc.sync.dma_start(out=b1s_t[:], in_=b1s[:])
            b2s_t = consts.tile([128, 1], dt.float32, tag="b2s_t")
            nc.sync.dma_start(out=b2s_t[:], in_=b2s[:])
            b3s_t = consts.tile([128, 1], dt.float32, tag="b3s_t")
            nc.sync.dma_start(out=b3s_t[:], in_=b3s[:])

            # Zero tiles driving the unmasked HAM warm-up matmuls.
            warm_w = consts.tile([128, 128], dt.float16, tag="warm_w")
            nc.vector.memset(warm_w[:], 0.0)
            warm_x = consts.tile([128, 256], dt.float16, tag="warm_x")
            nc.vector.memset(warm_x[:], 0.0)

            zpools = (pz0, pz1)

            xts = {}   # group -> xt tile
            ots = {}   # group -> output accumulation tile
            encs = {}  # (supertile, half) -> e tile
            h1ss = {}  # supertile -> h1s tile
            h2ss = {}  # supertile -> h2s tile

            # Initial HAM warm-up: ~5us of back-to-back full-array matmuls.
            # Masked (tile_position) matmuls never register as PE activity, so
            # without this the whole kernel runs at the cold 1.2 GHz clock.
            wz = pz0.tile([128, 2 * B], dt.float32, tag="z0", name="warmburst")
            for _ in range(24):
                nc.tensor.matmul(wz[:, 0:256], lhsT=warm_w[:], rhs=warm_x[:],
                                 start=True, stop=True)

            def ensure_group(g):
                if g in xts or g >= n_groups:
                    return
                g0 = g * GROUP_RAYS
                xt = xpool.tile([128, GROUP_RAYS], dt.float16, tag="xt",
                                name=f"xt{g}")
                nc.sync.dma_start(out=xt[:], in_=xp[:, g0 : g0 + GROUP_RAYS])
                xts[g] = xt
                ots[g] = outp.tile([128, B * G], dt.float16, tag="ot",
                                   name=f"ot{g}")

            def emit_expand_exp(t, half):
                """One 512-ray half: 2 full-array expand MMs + 1 Exp op.

                z layout [128, 1024]: cols 0:512 = enc rows 0:128 (hi lhsT,
                nonzero weight rows 0:37), cols 512:1024 = enc rows 128:224 +
                32 pad rows (lo lhsT, nonzero rows 64:101). Both matmuls use
                full [128, 128] lhsT with zero padding and the full-height xt
                (dead rows are zeros), so they're UNMASKED: this is what keeps
                the PE's HAM clock gate at 2.4 GHz -- masked (tile_position)
                matmuls never register as PE activity, and a cold PE runs
                everything at 1.2 GHz.
                """
                g, j = divmod(t, G)
                xt = xts[g]
                c0 = j * SUPER + half * B
                cols = slice(c0, c0 + B)
                z = zpools[half].tile([128, 2 * B], dt.float32,
                                      tag=f"z{half}", name=f"z{t}_{half}")
                e = encp.tile([128, 2 * B], dt.float16, tag="e",
                              name=f"e{t}_{half}")
                nc.tensor.matmul(
                    z[:, 0:B], lhsT=lw2_t[:, 0:128], rhs=xt[:, cols],
                    start=True, stop=True,
                )
                nc.tensor.matmul(
                    z[:, B : 2 * B], lhsT=lw2_t[:, 128:256], rhs=xt[:, cols],
                    start=True, stop=True,
                )
                nc.scalar.activation(e[:], z[:], Act.Exp)
                encs[(t, half)] = e

            def emit_l1(i):
                eA = encs.pop((i, 0))
                eB = encs.pop((i, 1))
                h1 = ph.tile([128, B], dt.float32, tag="hh", name=f"h1_{i}")
                nc.tensor.matmul(h1[0:64, :], lhsT=w1a_t[:], rhs=eA[:, 0:B],
                                 start=True, stop=False, tile_position=(0, 0))
                nc.tensor.matmul(h1[64:128, :], lhsT=w1a_t[:], rhs=eB[:, 0:B],
                                 start=True, stop=False, tile_position=(0, 64))
                nc.tensor.matmul(h1[0:64, :], lhsT=w1b_t[:],
                                 rhs=eA[0 : ENC - 128, B : 2 * B],
                                 start=False, stop=True, tile_position=(0, 0))
                nc.tensor.matmul(h1[64:128, :], lhsT=w1b_t[:],
                                 rhs=eB[0 : ENC - 128, B : 2 * B],
                                 start=False, stop=True, tile_position=(0, 64))
                h1s = hp.tile([128, B], dt.float16, tag="h1s", name=f"h1s{i}")
                nc.vector.tensor_scalar(h1s[:], h1[:], b1s_t[:], 0.0,
                                        Alu.add, Alu.max)
                h1ss[i] = h1s

            def emit_l2(i):
                h1s = h1ss.pop(i)
                h2 = ph.tile([128, B], dt.float32, tag="hh", name=f"h2_{i}")
                nc.tensor.matmul(h2[0:64, :], lhsT=w2s_t[0:64, :],
                                 rhs=h1s[0:64, :],
                                 start=True, stop=True, tile_position=(0, 0))
                nc.tensor.matmul(h2[64:128, :], lhsT=w2s_t[64:128, :],
                                 rhs=h1s[64:128, :],
                                 start=True, stop=True, tile_position=(64, 64))
                h2s = hp.tile([128, B], dt.float16, tag="h2s", name=f"h2s{i}")
                nc.vector.tensor_scalar(h2s[:], h2[:], b2s_t[:], 0.0,
                                        Alu.add, Alu.max)
                h2ss[i] = h2s

            def emit_l3(i):
                g, j = divmod(i, G)
                h2s = h2ss.pop(i)
                # Flipped quadrants: L3 occupies the (0,64)/(64,0) quadrants
                # so it runs concurrently with L2 of a later supertile, which
                # uses (0,0)/(64,64). Output rows are therefore [B-rays; A-rays].
                op = ph.tile([128, B], dt.float32, tag="hh", name=f"op{i}")
                nc.tensor.matmul(op[64:128, :], lhsT=w3s_t[0:64, :],
                                 rhs=h2s[0:64, :],
                                 start=True, stop=True, tile_position=(0, 64))
                nc.tensor.matmul(op[0:64, :], lhsT=w3s_t[64:128, :],
                                 rhs=h2s[64:128, :],
                                 start=True, stop=True, tile_position=(64, 0))
                nc.vector.tensor_scalar_add(ots[g][:, j * B : (j + 1) * B],
                                            op[:], b3s_t[:])
                if j == G - 1:
                    half = B * G
                    nc.sync.dma_start(out=yt[:, g * half : (g + 1) * half],
                                      in_=ots[g][:])
                    del xts[g], ots[g]

            # Pipeline: expand+exp(t) | L1(t-2) | L2(t-3) | L3(t-4).
            # Each DVE bounce (h1s, h2s) gets a full supertile of slack
            # before its consumer matmul, so DVE latency stays off the PE's
            # critical path (with L1/L2 in the same stage the PE idled ~0.9us
            # per supertile waiting on h1s).
            for t in range(n_super + 4):
                if t < n_super:
                    ensure_group(t // G)
                    if t % G == 2:
                        # Prefetch the next group's 2MB input DMA ~6
                        # supertiles (~13us) ahead; issuing it at the group
                        # boundary stalled the pipeline ~2.5us per group.
                        ensure_group(t // G + 1)
                    emit_expand_exp(t, 0)
                    emit_expand_exp(t, 1)
                if 0 <= t - 2 < n_super:
                    emit_l1(t - 2)
                if 0 <= t - 3 < n_super:
                    emit_l2(t - 3)
                if 0 <= t - 4 < n_super:
                    emit_l3(t - 4)

    nc.finalize()
    return nc


def _get_nc():
    key = (NC_RAYS, N_GROUPS)
    if key not in _BUILD_CACHE:
        _BUILD_CACHE[key] = _build_bass(*key)
    return _BUILD_CACHE[key]


def _f16_hilo(x64):
    """Exact hi/lo split: x ~= hi + lo with hi, lo fp16 (inputs are fp64)."""
    hi = x64.astype(np.float16)
    lo = (x64 - hi.astype(np.float64)).astype(np.float16)
    return hi, lo


def _pack_weights(W1, b1, W2, b2, W3, b3, centers):
    c = centers.astype(np.float64)  # [32]
    inv2s2 = 0.5 / (SIGMA * SIGMA)  # 512
    # Per enc-row dj (d = dj//32, j = dj%32):
    #   z = -inv2s2*x_d^2 + (2*inv2s2*c_j)*x_d - inv2s2*c_j^2
    wx = 2.0 * inv2s2 * c  # [32] coefficient on x
    wq = -inv2s2  # coefficient on q = x^2 (exact in fp16)
    wu = -inv2s2 * c * c  # [32] coefficient on 1

    wx_hi = wx.astype(np.float16)
    wx_lo = (wx - wx_hi.astype(np.float64)).astype(np.float16)
    wu_hi = wu.astype(np.float16)
    wu_lo = (wu - wu_hi.astype(np.float64)).astype(np.float16)

    L = np.zeros((KROWS, ENC), np.float16)
    for d in range(IN_DIMS):
        cols = slice(d * BINS, (d + 1) * BINS)
        L[d, cols] = wx_hi
        L[7 + d, cols] = wx_lo
        L[14 + d, cols] = wx_hi
        L[21 + d, cols] = np.float16(wq)
        L[28 + d, cols] = np.float16(wq)
    L[35, :] = np.tile(wu_hi, IN_DIMS)
    L[36, :] = np.tile(wu_lo, IN_DIMS)

    # lw [128, 256]: col block 0:128 = hi lhsT (enc rows 0:128; weight rows
    # 0:37, rest zero), col block 128:256 = lo lhsT (enc rows 128:224 + 32
    # benign pad rows repeating enc 128:160; weight rows 64:101 -- the xp dup
    # block -- rest zero). Full-height zero-padded lhsT keeps the expand
    # matmuls unmasked (see _build_bass docstring).
    Lpad = np.concatenate([L[:, 128:ENC], L[:, 128 : 128 + (256 - ENC)]], 1)
    lw = np.zeros((128, 256), np.float16)
    lw[0:KROWS, 0:128] = L[:, 0:128]
    lw[64 : 64 + KROWS, 128:256] = Lpad

    w1 = W1.astype(np.float16)
    packs = {
        "lw": lw,
        "w1a": np.ascontiguousarray(w1[0:128]),
        "w1b": np.ascontiguousarray(w1[128:ENC]),
        "w2s": np.concatenate([W2, W2], 0).astype(np.float16),
        "w3s": np.concatenate([W3, W3], 0).astype(np.float16),
        "b1s": np.concatenate([b1, b1], 0).astype(np.float32).reshape(128, 1),
        "b2s": np.concatenate([b2, b2], 0).astype(np.float32).reshape(128, 1),
        "b3s": np.concatenate([b3, b3], 0).astype(np.float32).reshape(128, 1),
    }
    return packs


def _pack_inputs(pos01, wi01, rough01):
    x = np.concatenate(
        [np.asarray(pos01), np.asarray(wi01), np.asarray(rough01)], axis=1
    ).astype(np.float32)
    np.clip(x, 0.0, 1.0, out=x)
    x64 = x.astype(np.float64)
    q64 = x64 * x64
    x_hi, x_lo = _f16_hilo(x64)
    q_hi, q_lo = _f16_hilo(q64)
    ones = np.ones((x.shape[0], 2), np.float16)
    P = np.concatenate([x_hi, x_hi, x_lo, q_hi, q_lo, ones], axis=1)  # [N, 37]
    Pt = np.ascontiguousarray(P.T)  # [37, N] fp16
    xp = np.zeros((128, x.shape[0]), np.float16)
    xp[0:KROWS] = Pt
    xp[64 : 64 + KROWS] = Pt
    return xp


def kernel(pos01, wi01, rough01, W1, b1, W2, b2, W3, b3, centers):
    global LAST_RESULTS
    import os

    from concourse.bass_utils import run_bass_kernel_spmd

    nc = _get_nc()

    xp = _pack_inputs(pos01, wi01, rough01)
    wpacks = _pack_weights(
        np.asarray(W1), np.asarray(b1), np.asarray(W2), np.asarray(b2),
        np.asarray(W3), np.asarray(b3), np.asarray(centers),
    )

    in_maps = []
    for c in range(N_CORES):
        m = dict(wpacks)
        m["xp"] = np.ascontiguousarray(xp[:, c * NC_RAYS : (c + 1) * NC_RAYS])
        in_maps.append(m)

    trace = bool(int(os.environ.get("KERNEL_TRACE", "0")))
    res = run_bass_kernel_spmd(nc, in_maps, list(range(N_CORES)), trace=trace)
    LAST_RESULTS = res

    out = np.empty((N_TOTAL, OUT), np.float32)
    for c in range(N_CORES):
        yt = res.results[c]["yt"]  # [128, NC_RAYS // 2] fp16
        arr = yt.reshape(128, N_GROUPS, G, B)
        # L3's flipped quadrants put A-half rays on rows 64:128, B on 0:64
        a = arr[OUT:128].transpose(1, 2, 3, 0)  # [g, j, r, 64]
        b = arr[0:OUT].transpose(1, 2, 3, 0)
        stacked = np.stack([a, b], axis=2)  # [g, j, 2, 512, 64]
        out[c * NC_RAYS : (c + 1) * NC_RAYS] = (
            stacked.reshape(NC_RAYS, OUT).astype(np.float32)
        )
    return out


# revision 17
# speedup vs baseline: 2.7652x; 1.0112x over previous
"""Trainium2 Bass kernel for a OneBlob-encoded 3-layer MLP (ConditioningNetwork).

Math:  x = clip(concat(pos01, wi01, rough01), 0, 1)          [N, 7]
       enc[n, d*32+j] = exp(-0.5 ((x[n,d]-c[j]) / sigma)^2)  [N, 224], sigma = 1/32
       y = relu(relu(enc@W1+b1)@W2+b2)@W3+b3                 [N, 64]

Strategy (pure data parallel over 8 cores, weights replicated):
  - The Gaussian exponent z = -(x-c)^2/(2 sigma^2) is affine in (x, x^2), so it is
    computed on the PE as one small matmul ("expand"):  z = L^T @ [x; x^2; 1].
    The quadratic has catastrophic cancellation near x == c, so x and x^2 are fed
    as exact fp16 hi+lo pairs and L carries hi/lo weight rows; all products are
    exact in fp16 (accumulated fp32 in PSUM) giving |dz| ~ 1e-3.
  - enc = Exp(z) on the scalar engine (ACT) straight out of PSUM, fp16 into SBUF.
  - Per 512-ray half, z is packed [128, 1024]: cols 0:512 = enc rows 0:128,
    cols 512:1024 = enc rows 128:224 (+ 32 benign pad rows so all 128 PSUM
    partitions are written). One ACT Exp op covers the whole half -> no idle
    ACT lanes and half-granular z double-buffering (2 tiles x 2 banks).
  - 3 MLP matmuls in fp16 (fp32 PSUM). A/B = consecutive 512-ray halves of a
    1024-ray supertile: L1 col-packed (psum partitions 0:64 / 64:128), L2/L3
    quadrant-packed (disjoint row+col groups -> concurrent on the PE).
    Bias+ReLU / bias+cast are single DVE tensor_scalar ops per [128, 512] tile.
  - The PE's HAM clock gate never warms on row/col-masked (tile_position)
    matmuls -- measured: they run at 1.2 GHz forever, but do run at 2.4 GHz
    once warm. So the kernel issues an unmasked full-array warm-up burst at
    start and one tiny unmasked dummy matmul per supertile (output is
    overwritten by L1's start accumulation) to keep the HAM at K=8/8.
  - Output is produced feature-major, packed [128, Nc/2] fp16 (per supertile:
    A-half rays on partitions 64:128, B-half on 0:64 -- L3 quadrant flip), and
    unpacked/transposed/cast on the host.

Input row packing (fp16, 37 rows, present twice in the [128, N] xp tensor:
rows 0:37 for the hi expand matmul, rows 64:101 for the lo one so the two
run concurrently on disjoint PE row groups; other rows dead -- the full
128-partition load is what spreads the DMA across all 16 SDMA engines):
  rows  0: 7  x_hi          (weights: w_hi,  w = c/sigma^2 per enc row)
  rows  7:14  x_hi (dup)    (weights: w_lo = w - fp16(w))
  rows 14:21  x_lo          (weights: w_hi)
  rows 21:28  q_hi, q = x^2 (weights: -1/(2 sigma^2), exact in fp16)
  rows 28:35  q_lo          (weights: -1/(2 sigma^2))
  row  35     ones          (weights: u_hi, u = -c^2/(2 sigma^2))
  row  36     ones          (weights: u_lo)
"""

import sys

import numpy as np

if "/opt/trn_rl_repo" not in sys.path:
    sys.path.insert(0, "/opt/trn_rl_repo")

N_CORES = 8
N_TOTAL = 1048576
NC_RAYS = N_TOTAL // N_CORES  # 131072 rays per core
BINS = 32
HID = 64
OUT = 64
IN_DIMS = 7
ENC = IN_DIMS * BINS  # 224
SIGMA = 1.0 / BINS

KROWS = 37  # packed input rows (see module docstring)
B = 512  # rays per matmul (one fp32 PSUM bank)
SUPER = 2 * B  # rays per supertile (A/B halves)
G = 8  # supertiles per DMA group
GROUP_RAYS = SUPER * G  # 8192
N_GROUPS = NC_RAYS // GROUP_RAYS  # 16

# Set by the last kernel() call so a test harness can read profile/exec time.
LAST_RESULTS = None

_BUILD_CACHE = {}


def _build_bass(nc_rays, n_groups):
    import concourse.tile as tile
    from concourse import bacc, mybir

    dt = mybir.dt
    Act = mybir.ActivationFunctionType
    Alu = mybir.AluOpType

    nc = bacc.Bacc("TRN2", target_bir_lowering=False, debug=False)

    n_super = n_groups * G

    # xp rows: 0:37 packed operand P, 64:101 duplicate of P (so hi/lo expand
    # matmuls stream from disjoint PE row groups and run concurrently); the
    # remaining rows are dead, padding the load to all 128 partitions so the
    # input DMA spreads across all 16 SDMA engines (non-128-partition loads
    # measured ~10us per 600KB -- they land on a fraction of the engines).
    xp = nc.dram_tensor("xp", [128, nc_rays], dt.float16, kind="ExternalInput")
    # lw [128, 256]: col block 0:128 = hi expand lhsT (rows 0:37 = packed
    # weights, rest zero), col block 128:256 = lo expand lhsT (rows 64:101 =
    # packed weights, rest zero). Full-height lhsT -> UNMASKED matmuls, which
    # are the only kind the PE's HAM clock gate counts as activity.
    lw = nc.dram_tensor("lw", [128, 256], dt.float16, kind="ExternalInput")
    w1a = nc.dram_tensor("w1a", [128, HID], dt.float16, kind="ExternalInput")
    w1b = nc.dram_tensor("w1b", [ENC - 128, HID], dt.float16, kind="ExternalInput")
    w2s = nc.dram_tensor("w2s", [128, HID], dt.float16, kind="ExternalInput")
    w3s = nc.dram_tensor("w3s", [128, OUT], dt.float16, kind="ExternalInput")
    b1s = nc.dram_tensor("b1s", [128, 1], dt.float32, kind="ExternalInput")
    b2s = nc.dram_tensor("b2s", [128, 1], dt.float32, kind="ExternalInput")
    b3s = nc.dram_tensor("b3s", [128, 1], dt.float32, kind="ExternalInput")
    # Output, packed fp16: per 512-col supertile block, rows 64:128 = A-half
    # rays, rows 0:64 = B-half rays (L3's flipped quadrants).
    yt = nc.dram_tensor("yt", [128, nc_rays // 2], dt.float16,
                        kind="ExternalOutput")

    with tile.TileContext(nc) as tc:
        with (
            tc.tile_pool(name="consts", bufs=1) as consts,
            tc.tile_pool(name="xpool", bufs=3) as xpool,
            tc.tile_pool(name="encp", bufs=8) as encp,
            tc.tile_pool(name="hp", bufs=4) as hp,
            tc.tile_pool(name="outp", bufs=3) as outp,
            tc.tile_pool(name="pz0", bufs=1, space="PSUM") as pz0,
            tc.tile_pool(name="pz1", bufs=1, space="PSUM") as pz1,
            tc.tile_pool(name="ph", bufs=4, space="PSUM") as ph,
        ):
            lw2_t = consts.tile([128, 256], dt.float16, tag="lw2_t")
            nc.sync.dma_start(out=lw2_t[:], in_=lw[:])
            w1a_t = consts.tile([128, HID], dt.float16, tag="w1a_t")
            nc.sync.dma_start(out=w1a_t[:], in_=w1a[:])
            w1b_t = consts.tile([ENC - 128, HID], dt.float16, tag="w1b_t")
            nc.sync.dma_start(out=w1b_t[:], in_=w1b[:])
            w2s_t = consts.tile([128, HID], dt.float16, tag="w2s_t")
            nc.sync.dma_start(out=w2s_t[:], in_=w2s[:])
            w3s_t = consts.tile([128, OUT], dt.float16, tag="w3s_t")
            nc.sync.dma_start(out=w3s_t[:], in_=w3s[:])
            b1s_t = consts.tile([128, 1], dt.float32, tag="b1s_t")
            nc.sync.dma_start(out=b1s_t[:], in_=b1s[:])
            b2s_t = consts.tile([128, 1], dt.float32, tag="b2s_t")
            nc.sync.dma_start(out=b2s_t[:], in_=b2s[:])
            b3s_t = consts.tile([128, 1], dt.float32, tag="b3s_t")
            nc.sync.dma_start(out=b3s_t[:], in_=b3s[:])

            # Zero tiles driving the unmasked HAM warm-up matmuls.
            warm_w = consts.tile([128, 128], dt.float16, tag="warm_w")
            nc.vector.memset(warm_w[:], 0.0)
            warm_x = consts.tile([128, 256], dt.float16, tag="warm_x")
            nc.vector.memset(warm_x[:], 0.0)

            zpools = (pz0, pz1)

            xts = {}   # group -> xt tile
            ots = {}   # group -> output accumulation tile
            encs = {}  # (supertile, half) -> e tile
            h1ss = {}  # supertile -> h1s tile
            h2ss = {}  # supertile -> h2s tile

            # Initial HAM warm-up: ~5us of back-to-back full-array matmuls.
            # Masked (tile_position) matmuls never register as PE activity, so
            # without this the whole kernel runs at the cold 1.2 GHz clock.
            wz = pz0.tile([128, 2 * B], dt.float32, tag="z0", name="warmburst")
            for _ in range(12):
                nc.tensor.matmul(wz[:, 0:256], lhsT=warm_w[:], rhs=warm_x[:],
                                 start=True, stop=True)

            def ensure_group(g):
                if g in xts or g >= n_groups:
                    return
                g0 = g * GROUP_RAYS
                xt = xpool.tile([128, GROUP_RAYS], dt.float16, tag="xt",
                                name=f"xt{g}")
                nc.sync.dma_start(out=xt[:], in_=xp[:, g0 : g0 + GROUP_RAYS])
                xts[g] = xt
                ots[g] = outp.tile([128, B * G], dt.float16, tag="ot",
                                   name=f"ot{g}")

            def emit_expand_exp(t, half):
                """One 512-ray half: 2 full-array expand MMs + 1 Exp op.

                z layout [128, 1024]: cols 0:512 = enc rows 0:128 (hi lhsT,
                nonzero weight rows 0:37), cols 512:1024 = enc rows 128:224 +
                32 pad rows (lo lhsT, nonzero rows 64:101). Both matmuls use
                full [128, 128] lhsT with zero padding and the full-height xt
                (dead rows are zeros), so they're UNMASKED: this is what keeps
                the PE's HAM clock gate at 2.4 GHz -- masked (tile_position)
                matmuls never register as PE activity, and a cold PE runs
                everything at 1.2 GHz.
                """
                g, j = divmod(t, G)
                xt = xts[g]
                c0 = j * SUPER + half * B
                cols = slice(c0, c0 + B)
                z = zpools[half].tile([128, 2 * B], dt.float32,
                                      tag=f"z{half}", name=f"z{t}_{half}")
                e = encp.tile([128, 2 * B], dt.float16, tag="e",
                              name=f"e{t}_{half}")
                nc.tensor.matmul(
                    z[:, 0:B], lhsT=lw2_t[:, 0:128], rhs=xt[:, cols],
                    start=True, stop=True,
                )
                nc.tensor.matmul(
                    z[:, B : 2 * B], lhsT=lw2_t[:, 128:256], rhs=xt[:, cols],
                    start=True, stop=True,
                )
                nc.scalar.activation(e[:], z[:], Act.Exp)
                encs[(t, half)] = e

            def emit_l1(i):
                eA = encs.pop((i, 0))
                eB = encs.pop((i, 1))
                h1 = ph.tile([128, B], dt.float32, tag="hh", name=f"h1_{i}")
                nc.tensor.matmul(h1[0:64, :], lhsT=w1a_t[:], rhs=eA[:, 0:B],
                                 start=True, stop=False, tile_position=(0, 0))
                nc.tensor.matmul(h1[64:128, :], lhsT=w1a_t[:], rhs=eB[:, 0:B],
                                 start=True, stop=False, tile_position=(0, 64))
                nc.tensor.matmul(h1[0:64, :], lhsT=w1b_t[:],
                                 rhs=eA[0 : ENC - 128, B : 2 * B],
                                 start=False, stop=True, tile_position=(0, 0))
                nc.tensor.matmul(h1[64:128, :], lhsT=w1b_t[:],
                                 rhs=eB[0 : ENC - 128, B : 2 * B],
                                 start=False, stop=True, tile_position=(0, 64))
                h1s = hp.tile([128, B], dt.float16, tag="h1s", name=f"h1s{i}")
                nc.vector.tensor_scalar(h1s[:], h1[:], b1s_t[:], 0.0,
                                        Alu.add, Alu.max)
                h1ss[i] = h1s

            def emit_l2(i):
                h1s = h1ss.pop(i)
                h2 = ph.tile([128, B], dt.float32, tag="hh", name=f"h2_{i}")
                nc.tensor.matmul(h2[0:64, :], lhsT=w2s_t[0:64, :],
                                 rhs=h1s[0:64, :],
                                 start=True, stop=True, tile_position=(0, 0))
                nc.tensor.matmul(h2[64:128, :], lhsT=w2s_t[64:128, :],
                                 rhs=h1s[64:128, :],
                                 start=True, stop=True, tile_position=(64, 64))
                h2s = hp.tile([128, B], dt.float16, tag="h2s", name=f"h2s{i}")
                nc.vector.tensor_scalar(h2s[:], h2[:], b2s_t[:], 0.0,
                                        Alu.add, Alu.max)
                h2ss[i] = h2s

            def emit_l3(i):
                g, j = divmod(i, G)
                h2s = h2ss.pop(i)
                # Flipped quadrants: L3 occupies the (0,64)/(64,0) quadrants
                # so it runs concurrently with L2 of a later supertile, which
                # uses (0,0)/(64,64). Output rows are therefore [B-rays; A-rays].
                op = ph.tile([128, B], dt.float32, tag="hh", name=f"op{i}")
                nc.tensor.matmul(op[64:128, :], lhsT=w3s_t[0:64, :],
                                 rhs=h2s[0:64, :],
                                 start=True, stop=True, tile_position=(0, 64))
                nc.tensor.matmul(op[0:64, :], lhsT=w3s_t[64:128, :],
                                 rhs=h2s[64:128, :],
                                 start=True, stop=True, tile_position=(64, 0))
                nc.vector.tensor_scalar_add(ots[g][:, j * B : (j + 1) * B],
                                            op[:], b3s_t[:])
                # Flush output per half-group (0.5MB) so the final store
                # starts 4 supertiles earlier -- trims the pipeline tail.
                if j == G // 2 - 1 or j == G - 1:
                    gbase = g * B * G
                    c0 = (j + 1 - G // 2) * B
                    c1 = (j + 1) * B
                    nc.sync.dma_start(out=yt[:, gbase + c0 : gbase + c1],
                                      in_=ots[g][:, c0:c1])
                if j == G - 1:
                    del xts[g], ots[g]

            # Pipeline: expand+exp(t) | L1(t-2) | L2(t-3) | L3(t-4).
            # Each DVE bounce (h1s, h2s) gets a full supertile of slack
            # before its consumer matmul, so DVE latency stays off the PE's
            # critical path (with L1/L2 in the same stage the PE idled ~0.9us
            # per supertile waiting on h1s).
            for t in range(n_super + 4):
                if t < n_super:
                    ensure_group(t // G)
                    if t % G == 2:
                        # Prefetch the next group's 2MB input DMA ~6
                        # supertiles (~13us) ahead; issuing it at the group
                        # boundary stalled the pipeline ~2.5us per group.
                        ensure_group(t // G + 1)
                    emit_expand_exp(t, 0)
                    emit_expand_exp(t, 1)
                if 0 <= t - 2 < n_super:
                    emit_l1(t - 2)
                if 0 <= t - 3 < n_super:
                    emit_l2(t - 3)
                if 0 <= t - 4 < n_super:
                    emit_l3(t - 4)

    nc.finalize()
    return nc


def _get_nc():
    key = (NC_RAYS, N_GROUPS)
    if key not in _BUILD_CACHE:
        _BUILD_CACHE[key] = _build_bass(*key)
    return _BUILD_CACHE[key]


def _f16_hilo(x64):
    """Exact hi/lo split: x ~= hi + lo with hi, lo fp16 (inputs are fp64)."""
    hi = x64.astype(np.float16)
    lo = (x64 - hi.astype(np.float64)).astype(np.float16)
    return hi, lo


def _pack_weights(W1, b1, W2, b2, W3, b3, centers):
    c = centers.astype(np.float64)  # [32]
    inv2s2 = 0.5 / (SIGMA * SIGMA)  # 512
    # Per enc-row dj (d = dj//32, j = dj%32):
    #   z = -inv2s2*x_d^2 + (2*inv2s2*c_j)*x_d - inv2s2*c_j^2
    wx = 2.0 * inv2s2 * c  # [32] coefficient on x
    wq = -inv2s2  # coefficient on q = x^2 (exact in fp16)
    wu = -inv2s2 * c * c  # [32] coefficient on 1

    wx_hi = wx.astype(np.float16)
    wx_lo = (wx - wx_hi.astype(np.float64)).astype(np.float16)
    wu_hi = wu.astype(np.float16)
    wu_lo = (wu - wu_hi.astype(np.float64)).astype(np.float16)

    L = np.zeros((KROWS, ENC), np.float16)
    for d in range(IN_DIMS):
        cols = slice(d * BINS, (d + 1) * BINS)
        L[d, cols] = wx_hi
        L[7 + d, cols] = wx_lo
        L[14 + d, cols] = wx_hi
        L[21 + d, cols] = np.float16(wq)
        L[28 + d, cols] = np.float16(wq)
    L[35, :] = np.tile(wu_hi, IN_DIMS)
    L[36, :] = np.tile(wu_lo, IN_DIMS)

    # lw [128, 256]: col block 0:128 = hi lhsT (enc rows 0:128; weight rows
    # 0:37, rest zero), col block 128:256 = lo lhsT (enc rows 128:224 + 32
    # benign pad rows repeating enc 128:160; weight rows 64:101 -- the xp dup
    # block -- rest zero). Full-height zero-padded lhsT keeps the expand
    # matmuls unmasked (see _build_bass docstring).
    Lpad = np.concatenate([L[:, 128:ENC], L[:, 128 : 128 + (256 - ENC)]], 1)
    lw = np.zeros((128, 256), np.float16)
    lw[0:KROWS, 0:128] = L[:, 0:128]
    lw[64 : 64 + KROWS, 128:256] = Lpad

    w1 = W1.astype(np.float16)
    packs = {
        "lw": lw,
        "w1a": np.ascontiguousarray(w1[0:128]),
        "w1b": np.ascontiguousarray(w1[128:ENC]),
        "w2s": np.concatenate([W2, W2], 0).astype(np.float16),
        "w3s": np.concatenate([W3, W3], 0).astype(np.float16),
        "b1s": np.concatenate([b1, b1], 0).astype(np.float32).reshape(128, 1),
        "b2s": np.concatenate([b2, b2], 0).astype(np.float32).reshape(128, 1),
        "b3s": np.concatenate([b3, b3], 0).astype(np.float32).reshape(128, 1),
    }
    return packs


def _pack_inputs(pos01, wi01, rough01):
    x = np.concatenate(
        [np.asarray(pos01), np.asarray(wi01), np.asarray(rough01)], axis=1
    ).astype(np.float32)
    np.clip(x, 0.0, 1.0, out=x)
    x64 = x.astype(np.float64)
    q64 = x64 * x64
    x_hi, x_lo = _f16_hilo(x64)
    q_hi, q_lo = _f16_hilo(q64)
    ones = np.ones((x.shape[0], 2), np.float16)
    P = np.concatenate([x_hi, x_hi, x_lo, q_hi, q_lo, ones], axis=1)  # [N, 37]
    Pt = np.ascontiguousarray(P.T)  # [37, N] fp16
    xp = np.zeros((128, x.shape[0]), np.float16)
    xp[0:KROWS] = Pt
    xp[64 : 64 + KROWS] = Pt
    return xp


def kernel(pos01, wi01, rough01, W1, b1, W2, b2, W3, b3, centers):
    global LAST_RESULTS
    import os

    from concourse.bass_utils import run_bass_kernel_spmd

    nc = _get_nc()

    xp = _pack_inputs(pos01, wi01, rough01)
    wpacks = _pack_weights(
        np.asarray(W1), np.asarray(b1), np.asarray(W2), np.asarray(b2),
        np.asarray(W3), np.asarray(b3), np.asarray(centers),
    )

    in_maps = []
    for c in range(N_CORES):
        m = dict(wpacks)
        m["xp"] = np.ascontiguousarray(xp[:, c * NC_RAYS : (c + 1) * NC_RAYS])
        in_maps.append(m)

    trace = bool(int(os.environ.get("KERNEL_TRACE", "0")))
    res = run_bass_kernel_spmd(nc, in_maps, list(range(N_CORES)), trace=trace)
    LAST_RESULTS = res

    out = np.empty((N_TOTAL, OUT), np.float32)
    for c in range(N_CORES):
        yt = res.results[c]["yt"]  # [128, NC_RAYS // 2] fp16
        arr = yt.reshape(128, N_GROUPS, G, B)
        # L3's flipped quadrants put A-half rays on rows 64:128, B on 0:64
        a = arr[OUT:128].transpose(1, 2, 3, 0)  # [g, j, r, 64]
        b = arr[0:OUT].transpose(1, 2, 3, 0)
        stacked = np.stack([a, b], axis=2)  # [g, j, 2, 512, 64]
        out[c * NC_RAYS : (c + 1) * NC_RAYS] = (
            stacked.reshape(NC_RAYS, OUT).astype(np.float32)
        )
    return out
